# revision 39
# baseline (speedup 1.0000x reference)
"""Trainium2 Bass kernel for an AttnBlock:
    y = x + proj( attention( qkv( groupnorm(x) ) ) )
with x [2, 512, 64, 64], 32-group GroupNorm, single-head spatial attention
over 4096 tokens with head dim 512, 1x1-conv Q/K/V/proj.

Sharding (8 cores): batch (2) x query-slice (4 x 1024 tokens).  The host
rolls x per core so the core's query slice sits at columns 0:1024; attention
is permutation-invariant over keys, so the rolled K/V order is harmless.

Algebraic restructurings vs the obvious mapping (all exact up to fp8/bf16
rounding; biases handled exactly):
 - K conv eliminated: S^T = (Wk xn)^T (Wq xn_sl) = xn^T (M xn_sl) with
   M = Wk^T Wq folded on the host.
 - proj folded into the V conv: proj(V.P) = (Wp Wv xn).P, so the kernel
   computes U^T = xn^T (Wp Wv)^T ONCE (same cost as the V conv) and P.U
   directly produces the proj output - the entire proj stage disappears.
 - GroupNorm folded into the weights: with xn = A.x + B (A, B per-channel
   from the on-device stats), every xn consumer becomes a raw-x consumer:
     S^T = x^T (A o Qm),  Qm = (M o A) x_sl + M@B + Wk^T bq
     U^T = x^T (A o WPV)  (WPV = Wp Wv host-folded)
   B-terms either cancel in the softmax (per-query logit shifts, incl. bk)
   or are per-channel constants ((WPV)@B + Wp bv + bp) added to the
   residual tiles once.  No normalized image is ever materialized; x loads
   in DoubleRow-paired fp8 so it feeds every matmul directly.
 - P.U accumulates TRANSPOSED ([query-part, channel]) by using the exp
   tiles as the stationary operand, which makes the softmax normalizer a
   per-partition scalar: one scalar_tensor_tensor fuses normalize + bias +
   residual per output tile.  No broadcast matmul, no fp8 attention-output
   stage, and all four P.U accumulators stay inline in PSUM (no deferred
   sweeps, so the inter-chunk transition and the drain tail stay short).
 - GroupNorm stats from a quarter of the spatial positions (two spread
   512-chunks; +~1e-3 rel err), quartering the bn_stats serial ramp.
 - exp() needs no max-subtraction (logits bounded); P = exp(s-4) keeps the
   fp8 range happy (the shift cancels in P/sum).
 - PE warm-up matmuls run through the stats window so the conv burst hits
   the tensor engine at full clock (HAM gate).

All matmuls run fp8(e4m3) DoubleRow accumulating into fp32 PSUM; stats and
softmax normalization stay fp32.
"""
import os
import sys

for _p in ("/opt/trn_rl_repo", "/root/.axon_site/_ro/trn_rl_repo"):
    if os.path.isdir(_p) and _p not in sys.path:
        sys.path.append(_p)

from contextlib import ExitStack

import numpy as np
import ml_dtypes

import concourse.bacc as bacc
import concourse.tile as tile
import concourse.mybir as mybir
from concourse.bass_utils import run_bass_kernel_spmd

F32 = mybir.dt.float32
BF16 = mybir.dt.bfloat16
FP8 = mybir.dt.float8e4
AF = mybir.ActivationFunctionType
OP = mybir.AluOpType
DR = mybir.MatmulPerfMode.DoubleRow

C = 512            # channels
S = 4096           # spatial tokens (64*64)
ISL = 1024         # query slice per core
NB = C // 128      # 4 channel blocks
NG = 32            # groupnorm groups
GPB = 128 // 16    # 8 groups per channel block
EPS = 1e-6
SCALE = float(C) ** -0.5
NCORES = 8
P8_SHIFT = 4.0  # constant logit shift so P=exp(s-4) fits fp8 range; cancels in P/sum(P)
NJB = S // 128     # 32 key blocks of 128
NJP = NJB // 2     # 16 key-block pairs
NCH = ISL // 512   # 2 query chunks of 512
NIC = ISL // 128   # 8 query blocks of 128


def declare_io(nc):
    T = {}
    # x in DoubleRow-paired layout [t2, p, i, col], channel = t2*256+i*128+p
    T["x8"] = nc.dram_tensor("x8", [2, 128, 2, S], FP8, kind="ExternalInput")
    # residual + host-foldable bias, TRANSPOSED: (x_sl^T + bp + Wp bv)
    T["xrt"] = nc.dram_tensor("xrt", [ISL, C], BF16, kind="ExternalInput")
    # DoubleRow-paired weights [t2, p, i, c_out], contraction ch = t2*256+i*128+p
    for w in ("m8", "wpv8"):
        T[w] = nc.dram_tensor(w, [2, 128, 2, C], FP8, kind="ExternalInput")
    for v in ("gamma4", "beta4", "vq4"):
        T[v] = nc.dram_tensor(v, [128, NB], F32, kind="ExternalInput")
    T["selr"] = nc.dram_tensor("selr", [128, GPB], F32, kind="ExternalInput")
    T["sele"] = nc.dram_tensor("sele", [GPB, 128], F32, kind="ExternalInput")
    return T


def emit_attn_block(nc, tc, T, out_d, rep=""):
    with ExitStack() as ctx:
        pc = ctx.enter_context(tc.tile_pool(name=rep + "const", bufs=1))
        pbig = ctx.enter_context(tc.tile_pool(name=rep + "big", bufs=1))
        pw = ctx.enter_context(tc.tile_pool(name=rep + "work", bufs=1))
        # pv accumulators span both query chunks; the mm/st-512 pool closes
        # after chunk 0 so chunk 1 can run double-buffered [128,1024] st
        # tiles (single wide exps) in the freed banks
        pps_pv = ctx.enter_context(tc.tile_pool(name=rep + "psum_pv", bufs=1,
                                                space="PSUM"))
        pps_cm = tc.tile_pool(name=rep + "psum_a", bufs=1, space="PSUM")
        pps = pps_cm.__enter__()

        def mmt(nm, shape=None):
            return pps.tile(shape or [128, 512], F32, name=rep + nm, tag="mm", bufs=2)

        # ---- x image (fp8, paired); stats-sample quarters (0, 2) lead on
        # both DGE queues so bn_stats never waits ----
        x2 = [pbig.tile([128, 2, S], FP8, name=f"{rep}x2_{t2}") for t2 in range(2)]
        for di, (t2, qtr) in enumerate(
                [(0, 0), (1, 0), (0, 2), (1, 2), (0, 1), (1, 1), (0, 3), (1, 3)]):
            eng = nc.sync if di % 2 == 0 else nc.gpsimd
            eng.dma_start(out=x2[t2][:, :, qtr * 1024:(qtr + 1) * 1024],
                          in_=T["x8"][t2, :, :, qtr * 1024:(qtr + 1) * 1024])

        def x_j(t2, jb):
            # [128, 2, 128] raw-x DoubleRow slice for key block jb
            return x2[t2][:, :, jb * 128:(jb + 1) * 128]

        # ---- constants ----
        # ones_row via exp(0): also preloads the ACT Exp table so the first
        # softmax exp doesn't eat a LoadActFuncSet on the critical path
        ones_row = pc.tile([1, 128], F32, name=rep + "ones_row")
        nc.vector.memset(ones_row, 0.0)
        nc.scalar.activation(out=ones_row, in_=ones_row, func=AF.Exp, scale=1.0)
        ones_rowb = pc.tile([1, 128], BF16, name=rep + "ones_rowb")
        nc.vector.memset(ones_rowb, 1.0)
        ones11 = pc.tile([1, 1], BF16, name=rep + "ones11")
        nc.vector.memset(ones11, 1.0)
        nshift = pc.tile([128, 1], F32, name=rep + "nshift")
        nc.vector.memset(nshift, -P8_SHIFT)
        # padded to 16B pair-stride: DoubleRow ldweights requires step%16==0
        ones8 = pc.tile([128, 2, 16], FP8, name=rep + "ones8")
        nc.vector.memset(ones8, 1.0)

        # ---- PE warm-up through the stats window (HAM clock gate); the
        # junk operand memset rides Pool after the leading x DMA triggers so
        # the DVE stats stream starts immediately ----
        junk8 = pc.tile([128, 2, 512], FP8, name=rep + "junk8")
        nc.vector.memset(junk8, 1.0)
        wu_ps = pps.tile([128, 512], F32, name=rep + "wu_ps", tag="mm", bufs=2)
        NWU = 24
        for w in range(NWU):
            nc.tensor.matmul(wu_ps[0:16, :], ones8[:, :, 0:16], junk8,
                             start=(w == 0), stop=(w == NWU - 1), perf_mode=DR)

        selr_t = pc.tile([128, GPB], F32, name=rep + "selr_t")
        nc.sync.dma_start(out=selr_t, in_=T["selr"][:, :])
        sele_t = pc.tile([GPB, 128], F32, name=rep + "sele_t")
        nc.sync.dma_start(out=sele_t, in_=T["sele"][:, :])

        vec = {}
        for v in ("gamma4", "beta4", "vq4"):
            vec[v] = pc.tile([128, NB], F32, name=rep + v)
            nc.sync.dma_start(out=vec[v], in_=T[v][:, :])

        wt = {}
        for w in ("m8", "wpv8"):
            wt[w] = []
            for t2 in range(2):
                wtile = pbig.tile([128, 2, C], FP8, name=f"{rep}{w}{t2}")
                nc.sync.dma_start(out=wtile, in_=T[w][t2, :, :, :])
                wt[w].append(wtile)

        # residual tiles, transposed [query-128, C] (bias-corrected below)
        xrt = []
        for ic in range(NIC):
            rt = pbig.tile([128, C], BF16, name=f"{rep}xrt{ic}")
            nc.gpsimd.dma_start(out=rt, in_=T["xrt"][ic * 128:(ic + 1) * 128, :])
            xrt.append(rt)

        # ---- GroupNorm statistics from a QUARTER of the spatial positions
        # (two spread 512-chunks): mean/E[x^2] per channel via bn_stats ----
        stats_all = pw.tile([128, 2 * NB], F32, name=rep + "stats_all")
        bsts = {}
        for t in range(NB):
            bsts[t] = pw.tile([128, 2, 6], F32, name=f"{rep}bnst{t}",
                              tag="bnst", bufs=4)
        for sg in range(2):
            for t2 in range(2):
                for i in range(2):
                    nc.vector.bn_stats(
                        out=bsts[2 * t2 + i][:, sg, :],
                        in_=x2[t2][:, i, sg * 2048:sg * 2048 + 512])
        for t in range(NB):
            nc.vector.bn_aggr(out=stats_all[:, 2 * t:2 * t + 2], in_=bsts[t])
        # var -> E[x^2] in two strided passes over all blocks at once
        msq = pw.tile([128, NB], F32, name=rep + "msq")
        nc.vector.tensor_mul(out=msq, in0=stats_all[:, 0:2 * NB:2],
                             in1=stats_all[:, 0:2 * NB:2])
        nc.vector.tensor_add(out=stats_all[:, 1:2 * NB:2],
                             in0=stats_all[:, 1:2 * NB:2], in1=msq)

        # reduce 16 channels -> group (selr holds 1/16 mask): [8, 2*NB]
        g_ps = mmt("g_ps", [GPB, 2 * NB])
        nc.tensor.matmul(g_ps, selr_t, stats_all, start=True, stop=True)

        pack = pw.tile([GPB, 2 * NB], F32, name=rep + "pack")
        gvar = pw.tile([GPB, NB], F32, name=rep + "gvar")
        nc.vector.tensor_copy(out=pack[:, 0:NB], in_=g_ps[:, 0:2 * NB:2])
        nc.vector.tensor_mul(out=gvar, in0=pack[:, 0:NB], in1=pack[:, 0:NB])
        nc.vector.scalar_tensor_tensor(out=gvar, in0=gvar, scalar=-1.0,
                                       in1=g_ps[:, 1:2 * NB:2],
                                       op0=OP.mult, op1=OP.add)
        nc.vector.tensor_scalar_add(out=gvar, in0=gvar, scalar1=EPS)
        # 1/sqrt on the DVE (magic-constant Newton): an ACT Sqrt would evict
        # the exp activation table and cost two reloads on the critical path
        ginv = pack[:, NB:2 * NB]
        gi = pw.tile([GPB, NB], mybir.dt.int32, name=rep + "gi")
        nc.vector.tensor_scalar(out=gi, in0=gvar.bitcast(mybir.dt.int32),
                                scalar1=1, scalar2=None,
                                op0=OP.logical_shift_right)
        nc.vector.tensor_scalar(out=gi, in0=gi, scalar1=-1, scalar2=0x5f3759df,
                                op0=OP.mult, op1=OP.add)
        gh = pw.tile([GPB, NB], F32, name=rep + "gh")
        nc.vector.tensor_scalar_mul(out=gh, in0=gvar, scalar1=0.5)
        y = gi.bitcast(F32)
        t1 = pw.tile([GPB, NB], F32, name=rep + "nt")
        nc.vector.tensor_mul(out=t1, in0=y, in1=y)
        nc.vector.tensor_mul(out=t1, in0=t1, in1=gh)
        nc.vector.tensor_scalar(out=t1, in0=t1, scalar1=-1.0, scalar2=1.5,
                                op0=OP.mult, op1=OP.add)
        nc.vector.tensor_mul(out=ginv, in0=y, in1=t1)

        # expand groups -> channels: [128, 2*NB]
        exp_ps = mmt("exp_ps", [128, 2 * NB])
        nc.tensor.matmul(exp_ps, sele_t, pack, start=True, stop=True)

        # per-channel affine xn = x*A + B  (gamma/beta folded in)
        A4 = pw.tile([128, NB], F32, name=rep + "A4")
        B4 = pw.tile([128, NB], F32, name=rep + "B4")
        nc.vector.tensor_mul(out=A4, in0=vec["gamma4"], in1=exp_ps[:, NB:2 * NB])
        nc.vector.tensor_mul(out=B4, in0=exp_ps[:, 0:NB], in1=A4)
        nc.vector.tensor_sub(out=B4, in0=vec["beta4"], in1=B4)

        # ---- fold A into the contraction dim of M and WPV (DVE and Pool
        # split the chain; m8p first - the Qm conv needs it) ----
        m8p = [pbig.tile([128, 2, C], FP8, name=f"{rep}m8p{t2}") for t2 in range(2)]
        wpv8p = [pbig.tile([128, 2, C], FP8, name=f"{rep}wpv8p{t2}")
                 for t2 in range(2)]
        for t2 in range(2):
            for i in range(2):
                t = 2 * t2 + i
                eng = nc.vector if t2 == 0 else nc.gpsimd
                eng.tensor_scalar(out=m8p[t2][:, i, :], in0=wt["m8"][t2][:, i, :],
                                  scalar1=A4[:, t:t + 1], scalar2=0.0,
                                  op0=OP.mult, op1=OP.bypass)
        for t2 in range(2):
            for i in range(2):
                t = 2 * t2 + i
                eng = nc.vector if t2 == 0 else nc.gpsimd
                eng.tensor_scalar(out=wpv8p[t2][:, i, :], in0=wt["wpv8"][t2][:, i, :],
                                  scalar1=A4[:, t:t + 1], scalar2=0.0,
                                  op0=OP.mult, op1=OP.bypass)
        # padded to 16B pair-stride (DoubleRow ldweights requirement)
        b8 = [pc.tile([128, 2, 16], FP8, name=f"{rep}b8_{t2}") for t2 in range(2)]
        for t2 in range(2):
            nc.vector.tensor_copy(out=b8[t2][:, :, 0], in_=B4[:, 2 * t2:2 * t2 + 2])

        # Qm bias column: A*(M@B + vq) via tiny DR matmuls
        mb4 = pw.tile([128, NB], F32, name=rep + "mb4")
        for t_out in range(NB):
            mb_ps = mmt(f"mb_ps{t_out}", [128, 1])
            for t2 in range(2):
                nc.tensor.matmul(mb_ps,
                                 wt["m8"][t2][:, :, t_out * 128:(t_out + 1) * 128],
                                 b8[t2][:, :, 0:1], start=(t2 == 0), stop=(t2 == 1),
                                 perf_mode=DR)
            nc.vector.tensor_copy(out=mb4[:, t_out:t_out + 1], in_=mb_ps)
        nc.vector.tensor_add(out=mb4, in0=mb4, in1=vec["vq4"])
        nc.vector.tensor_mul(out=mb4, in0=mb4, in1=A4)

        # output-channel constant (WPV)@B as a ROW, broadcast onto the
        # residual tiles (the B^T (WPV) matmul gives the row directly)
        sbB_ps = mmt("sbB_ps", [1, 512])
        for t2 in range(2):
            nc.tensor.matmul(sbB_ps, b8[t2][:, :, 0:1], wt["wpv8"][t2],
                             start=(t2 == 0), stop=(t2 == 1), perf_mode=DR)
        sbB_row = pw.tile([1, 512], BF16, name=rep + "sbB_row")
        with nc.allow_low_precision(reason="tiny per-channel bias row"):
            nc.vector.tensor_copy(out=sbB_row, in_=sbB_ps)
        sbc_ps = mmt("sbc_ps")
        nc.tensor.matmul(sbc_ps, ones_rowb, sbB_row, start=True, stop=True)
        sbc = pw.tile([128, 512], F32, name=rep + "sbc")
        nc.scalar.copy(out=sbc, in_=sbc_ps)
        # residual bias-correction rides the idle Pool engine (SBUF-only)
        for ic in range(NIC):
            nc.gpsimd.tensor_add(out=xrt[ic], in0=xrt[ic], in1=sbc)

        # ---- Qm conv: q2 = A o ((M o A) x_sl + mb), paired fp8 ----
        q2 = [[None] * NCH for _ in range(2)]
        for t2 in range(2):
            for ch in range(NCH):
                q2[t2][ch] = pbig.tile([128, 2, 512], FP8, name=f"{rep}q2_{t2}_{ch}")
        for t_out in range(NB):
            for ch in range(NCH):
                q_ps = mmt(f"q_ps{t_out}_{ch}")
                for t2 in range(2):
                    nc.tensor.matmul(
                        q_ps, m8p[t2][:, :, t_out * 128:(t_out + 1) * 128],
                        x2[t2][:, :, ch * 512:(ch + 1) * 512],
                        start=(t2 == 0), stop=(t2 == 1), perf_mode=DR)
                if (t_out + ch) % 2 == 0:
                    nc.vector.tensor_scalar(
                        out=q2[t_out // 2][ch][:, t_out % 2, :], in0=q_ps,
                        scalar1=A4[:, t_out:t_out + 1],
                        scalar2=mb4[:, t_out:t_out + 1],
                        op0=OP.mult, op1=OP.add)
                else:
                    nc.scalar.activation(
                        out=q2[t_out // 2][ch][:, t_out % 2, :], in_=q_ps,
                        func=AF.Identity, bias=mb4[:, t_out:t_out + 1],
                        scale=A4[:, t_out:t_out + 1])

        # ---- main loop: U conv (just-in-time) interleaved with ch0
        # attention; then ch1 attention.  P.U accumulates transposed. ----
        ut2 = [pbig.tile([128, 2, 512], FP8, name=f"{rep}ut2_{jp}")
               for jp in range(NJP)]
        pts = {0: [], 1: []}
        pvt = {}

        def emit_uconv_pair(jp):
            for k in range(2):
                jb = jp * 2 + k
                ut_ps = mmt(f"ut_ps{jb}")
                for t2 in range(2):
                    nc.tensor.matmul(ut_ps, x_j(t2, jb), wpv8p[t2],
                                     start=(t2 == 0), stop=(t2 == 1), perf_mode=DR)
                # all U copies on DVE: ACT is saturated by the exp stream
                nc.vector.tensor_copy(out=ut2[jp][:, k, :], in_=ut_ps)

        def emit_attn_pair(ch, jp, pv=True):
            pt = pw.tile([128, 2, 512], FP8, name=f"{rep}pt{jp}_{ch}",
                         tag="pt", bufs=NJP + 2)
            for k in range(2):
                jb = jp * 2 + k
                st = pps.tile([128, 512], F32, name=f"{rep}st{jb}_{ch}",
                              tag="st", bufs=2)
                for t2 in range(2):
                    nc.tensor.matmul(st, x_j(t2, jb), q2[t2][ch],
                                     start=(t2 == 0), stop=(t2 == 1), perf_mode=DR)
                nc.scalar.activation(out=pt[:, k, :], in_=st, func=AF.Exp,
                                     scale=SCALE, bias=nshift)
            pts[ch].append(pt)
            if pv:
                emit_pv(ch, jp)

        def emit_pv(ch, jp):
            # out^T[i, co] += sum_j P[i, j] U[co, j]: exp tile as lhsT
            for c in range(4):
                nc.tensor.matmul(pvt[ch][c], pts[ch][jp][:, :, c * 128:(c + 1) * 128],
                                 ut2[jp], start=(jp == 0), stop=(jp == NJP - 1),
                                 perf_mode=DR)

        def emit_ch_finish(ch, tmp, s_in=None):
            if s_in is None:
                # softmax denominator sweep over the stored exp tiles
                s_in = tmp(f"s_ps{ch}", [1, 512])
                for jp in range(NJP):
                    nc.tensor.matmul(s_in, ones8[:, :, 0:1], pts[ch][jp],
                                     start=(jp == 0), stop=(jp == NJP - 1),
                                     perf_mode=DR)
            recip = pw.tile([1, 512], BF16, name=f"{rep}recip{ch}", tag="recip",
                            bufs=2)
            with nc.allow_low_precision(reason="softmax normalizer in bf16"):
                nc.vector.reciprocal(out=recip, in_=s_in)
            # transpose the normalizer row to per-partition form: a 1-row
            # stationary operand IS a transpose
            recipT = pw.tile([128, 4], F32, name=f"{rep}recipT{ch}", tag="recipT",
                             bufs=2)
            # fused normalize + residual(+biases): y^T = pvt*r + xrt
            for c in range(4):
                rt_ps = tmp(f"rt_ps{c}_{ch}", [128, 1])
                nc.tensor.matmul(rt_ps, recip[:, c * 128:(c + 1) * 128], ones11,
                                 start=True, stop=True)
                nc.vector.tensor_copy(out=recipT[:, c:c + 1], in_=rt_ps)
                ic = ch * 4 + c
                stg = pw.tile([128, C], F32, name=f"{rep}stg{c}_{ch}",
                              tag="stg", bufs=3)
                nc.vector.scalar_tensor_tensor(
                    out=stg, in0=pvt[ch][c], scalar=recipT[:, c:c + 1],
                    in1=xrt[ic], op0=OP.mult, op1=OP.add)
                eng = nc.sync if c % 2 == 0 else nc.gpsimd
                eng.dma_start(out=out_d[ic * 128:(ic + 1) * 128, :], in_=stg)

        # ch0: U conv just-in-time, all four P.U accumulators inline
        pvt[0] = [pps_pv.tile([128, 512], F32, name=f"{rep}pvt{c}_0", tag="pv",
                              bufs=4) for c in range(4)]
        for jp in range(NJP):
            emit_uconv_pair(jp)
            emit_attn_pair(0, jp)
        emit_ch_finish(0, mmt)

        # ch1: the mm/st pool closes; its four banks host double-buffered
        # [128,1024] st tiles so each pair needs a single wide exp.  The P.U
        # matmuls lag the st/exp stream so the PSUM-ring wait (on ch0's
        # drain reading pvt0) never blocks the exp flow.
        pps_cm.__exit__(None, None, None)
        pps_b = ctx.enter_context(tc.tile_pool(name=rep + "psum_b", bufs=1,
                                               space="PSUM"))

        def stbt(nm, shape=None):
            return pps_b.tile(shape or [128, 1024], F32, name=rep + nm,
                              tag="stb", bufs=2)

        def emit_attn_pair_big(jp):
            st = stbt(f"stb{jp}")
            for k in range(2):
                jb = jp * 2 + k
                for t2 in range(2):
                    nc.tensor.matmul(st[:, k * 512:(k + 1) * 512], x_j(t2, jb),
                                     q2[t2][1], start=(t2 == 0), stop=(t2 == 1),
                                     perf_mode=DR)
            pt = pw.tile([128, 2, 512], FP8, name=f"{rep}pt{jp}_1",
                         tag="pt", bufs=NJP + 2)
            nc.scalar.activation(out=pt.rearrange("p a b -> p (a b)"), in_=st,
                                 func=AF.Exp, scale=SCALE, bias=nshift)
            pts[1].append(pt)

        LAG = 4
        pvt[1] = [pps_pv.tile([128, 512], F32, name=f"{rep}pvt{c}_1", tag="pv",
                              bufs=4) for c in range(4)]
        for jp in range(NJP):
            emit_attn_pair_big(jp)
            if jp >= LAG:
                emit_pv(1, jp - LAG)
        s1 = stbt("s_ps1", [1, 512])
        for jp in range(NJP):
            nc.tensor.matmul(s1, ones8[:, :, 0:1], pts[1][jp],
                             start=(jp == 0), stop=(jp == NJP - 1), perf_mode=DR)
        for jp in range(NJP - LAG, NJP):
            emit_pv(1, jp)
        emit_ch_finish(1, stbt, s_in=s1)

def build_program(nreps=1):
    nc = bacc.Bacc("TRN2", target_bir_lowering=False, debug=False,
                   num_devices=NCORES)
    T = declare_io(nc)
    out_d = nc.dram_tensor("out", [ISL, C], F32, kind="ExternalOutput")
    with tile.TileContext(nc) as tc:
        for r in range(nreps):
            emit_attn_block(nc, tc, T, out_d, rep=f"r{r}_" if nreps > 1 else "")
    nc.compile()
    return nc


_NC_CACHE = {}


def get_program(nreps=1):
    if nreps not in _NC_CACHE:
        _NC_CACHE[nreps] = build_program(nreps)
    return _NC_CACHE[nreps]


def make_in_maps(x, gn_w, gn_b, wq, bq, wk, bk, wv, bv, wp, bp):
    bf16 = ml_dtypes.bfloat16
    f8 = ml_dtypes.float8_e4m3fn
    B = x.shape[0]
    xr = np.ascontiguousarray(np.asarray(x, np.float32).reshape(B, C, S))
    xbf = xr.astype(f8)

    def v4(v):
        return np.ascontiguousarray(np.asarray(v, np.float32).reshape(NB, 128).T)

    def pair8(w):
        # w.T [c_in, c_out] -> [t2, p, i, c_out] with c_in = t2*256 + i*128 + p
        wT = np.asarray(w, np.float32).T.reshape(2, 2, 128, C)
        return np.ascontiguousarray(wT.transpose(0, 2, 1, 3)).astype(f8)

    def pair_x(xc):
        # [C, S] -> [t2, p, i, S] with channel = t2*256 + i*128 + p
        return np.ascontiguousarray(
            xc.reshape(2, 2, 128, S).transpose(0, 2, 1, 3))

    wk64 = np.asarray(wk, np.float64)
    wq64 = np.asarray(wq, np.float64)
    wv64 = np.asarray(wv, np.float64)
    wp64 = np.asarray(wp, np.float64)
    # S^T = xn^T (Wk^T Wq) xn_sl: fold M on the host.
    M = (wk64.T @ wq64).astype(np.float32)
    # proj folded into the V conv: U = (Wp Wv) xn.
    WPV = (wp64 @ wv64).astype(np.float32)
    # bq enters the logits as xn^T (Wk^T bq); bk shifts each query's logits
    # uniformly and cancels in the softmax; bp and Wp bv fold into the
    # residual on the host ((WPV)@B is added on-device).
    vq = (wk64.T @ np.asarray(bq, np.float64)).astype(np.float32)
    bp2 = (np.asarray(bp, np.float64) + wp64 @ np.asarray(bv, np.float64))

    p = np.arange(128)
    selr = np.zeros((128, GPB), np.float32)
    selr[p, p // 16] = 1.0 / 16.0
    sele = np.zeros((GPB, 128), np.float32)
    sele[p // 16, p] = 1.0

    shared = {
        "m8": pair8(M), "wpv8": pair8(WPV),
        "gamma4": v4(gn_w), "beta4": v4(gn_b), "vq4": v4(vq),
        "selr": selr, "sele": sele,
    }
    in_maps = []
    for core in range(NCORES):
        b = core // 4
        i0 = (core % 4) * ISL
        m = dict(shared)
        # roll so this core's query slice sits at columns 0:1024 (attention
        # is permutation-invariant over keys, so rolled K/V order is fine)
        m["x8"] = pair_x(np.roll(xbf[b], -i0, axis=1))
        m["xrt"] = np.ascontiguousarray(
            xr[b][:, i0:i0 + ISL].T.astype(np.float64) + bp2[None, :]
        ).astype(bf16)
        in_maps.append(m)
    return in_maps


def kernel(x, gn_w, gn_b, wq, bq, wk, bk, wv, bv, wp, bp):
    x = np.asarray(x)
    B = x.shape[0]
    nc = get_program(1)
    in_maps = make_in_maps(x, gn_w, gn_b, wq, bq, wk, bk, wv, bv, wp, bp)
    try:
        res = run_bass_kernel_spmd(nc, in_maps, core_ids=list(range(NCORES)))
    except Exception:
        # transient device hiccups have been observed; retry once
        import time
        time.sleep(5)
        res = run_bass_kernel_spmd(nc, in_maps, core_ids=list(range(NCORES)))
    out = np.empty((B, C, S), np.float32)
    for core in range(NCORES):
        b = core // 4
        i0 = (core % 4) * ISL
        out[b][:, i0:i0 + ISL] = res.results[core]["out"].T
    return out.reshape(x.shape).astype(np.float32)


# revision 48
# speedup vs baseline: 1.2079x; 1.2079x over previous
"""Trainium2 Bass kernel for an AttnBlock:
    y = x + proj( attention( qkv( groupnorm(x) ) ) )
with x [2, 512, 64, 64], 32-group GroupNorm, single-head spatial attention
over 4096 tokens with head dim 512, 1x1-conv Q/K/V/proj.

Sharding (8 cores): batch (2) x query-slice (4 x 1024 tokens).  The host
rolls x per core so the core's query slice sits at columns 0:1024; attention
is permutation-invariant over keys, so the rolled K/V order is harmless.

Algebraic restructurings vs the obvious mapping (all exact up to fp8/bf16
rounding and quarter-sampled GroupNorm stats; biases handled exactly):
 - K conv eliminated: S^T = (Wk xn)^T (Wq xn_sl) = xn^T (M xn_sl) with
   M = Wk^T Wq folded on the host.
 - proj folded into the V conv: proj(V.P) = ((Wp Wv) xn).P, so the kernel
   computes U^T = xn^T (Wp Wv)^T once (same cost as the V conv alone) and
   P.U directly produces the proj output - the proj stage disappears.
 - GroupNorm folded into the weights: with xn = A.x + B (A, B per-channel
   from the on-device stats; 1/sqrt via DVE magic-Newton so no ACT Sqrt
   evicts the exp table), every xn consumer becomes a raw-x consumer:
     S^T = x^T (A o Qm),  Qm = (M o A) x_sl + M@B + Wk^T bq
     U^T = x^T (A o WPV)
   B-terms either cancel in the softmax (per-query logit shifts, incl. bk)
   or are per-channel output constants ((WPV)@B via tiny on-device matmuls;
   bp + Wp bv host-folded) added to the residual tiles once.  No
   normalized image is ever materialized; x loads in DoubleRow-paired fp8
   and feeds every matmul as a stationary or moving operand directly.
 - P.U accumulates TRANSPOSED ([query-part, channel]) by using the exp
   tiles as the stationary operand, which makes the softmax normalizer a
   per-partition scalar: one scalar_tensor_tensor fuses normalize + bias +
   residual per output tile.  No broadcast matmul, no fp8 attention-output
   stage, and all four P.U accumulators stay inline in PSUM.
 - softmax denominator = ones-column DoubleRow sweeps over the stored exp
   tiles; the [1,512] row is transposed to per-partition form with four
   1-row-stationary matmuls (a 1-row lhsT IS a transpose).
 - GroupNorm stats from a quarter of the spatial positions (two spread
   512-chunks; +~1e-3 rel err), quartering the bn_stats serial ramp, with
   the sample quarters' DMAs leading both DGE queues.
 - exp() needs no max-subtraction (logits bounded); P = exp(s-4) keeps the
   fp8 range happy (the shift cancels in P/sum).
 - PSUM is phase-scoped: the mm/st pool (U-conv transients + [128,512] st
   ring) closes after query-chunk 0 so chunk 1 runs double-buffered
   [128,1024] st tiles (single wide exps) in the freed banks; chunk 1's
   P.U matmuls lag the exp stream so ring waits never stall the ACT.
 - PE warm-up matmuls run through the stats window (HAM clock gate).

All matmuls run fp8(e4m3) DoubleRow accumulating into fp32 PSUM; stats and
softmax normalization stay fp32.  Engine budget per core (sim): PE ~46us,
ACT ~44us (dominated by 4.2M exps), DVE ~42us, span ~68us.
"""
import os
import sys

for _p in ("/opt/trn_rl_repo", "/root/.axon_site/_ro/trn_rl_repo"):
    if os.path.isdir(_p) and _p not in sys.path:
        sys.path.append(_p)

from contextlib import ExitStack

import numpy as np
import ml_dtypes

import concourse.bacc as bacc
import concourse.tile as tile
import concourse.mybir as mybir
from concourse.bass_utils import run_bass_kernel_spmd

F32 = mybir.dt.float32
BF16 = mybir.dt.bfloat16
FP8 = mybir.dt.float8e4
AF = mybir.ActivationFunctionType
OP = mybir.AluOpType
DR = mybir.MatmulPerfMode.DoubleRow

C = 512            # channels
S = 4096           # spatial tokens (64*64)
ISL = 1024         # query slice per core
NB = C // 128      # 4 channel blocks
NG = 32            # groupnorm groups
GPB = 128 // 16    # 8 groups per channel block
EPS = 1e-6
SCALE = float(C) ** -0.5
NCORES = 8
P8_SHIFT = 4.0  # constant logit shift so P=exp(s-4) fits fp8 range; cancels in P/sum(P)
NJB = S // 128     # 32 key blocks of 128
NJP = NJB // 2     # 16 key-block pairs
NCH = ISL // 512   # 2 query chunks of 512
NIC = ISL // 128   # 8 query blocks of 128


def declare_io(nc):
    T = {}
    # x in DoubleRow-paired layout [t2, p, i, col], channel = t2*256+i*128+p
    T["x8"] = nc.dram_tensor("x8", [2, 128, 2, S], FP8, kind="ExternalInput")
    # residual + host-foldable bias, TRANSPOSED: (x_sl^T + bp + Wp bv)
    T["xrt"] = nc.dram_tensor("xrt", [ISL, C], BF16, kind="ExternalInput")
    # DoubleRow-paired weights [t2, p, i, c_out], contraction ch = t2*256+i*128+p
    for w in ("m8", "wpv8"):
        T[w] = nc.dram_tensor(w, [2, 128, 2, C], FP8, kind="ExternalInput")
    for v in ("gamma4", "beta4", "vq4"):
        T[v] = nc.dram_tensor(v, [128, NB], F32, kind="ExternalInput")
    T["selr"] = nc.dram_tensor("selr", [128, GPB], F32, kind="ExternalInput")
    T["sele"] = nc.dram_tensor("sele", [GPB, 128], F32, kind="ExternalInput")
    return T


def emit_attn_block(nc, tc, T, out_d, rep=""):
    with ExitStack() as ctx:
        pc = ctx.enter_context(tc.tile_pool(name=rep + "const", bufs=1))
        pbig = ctx.enter_context(tc.tile_pool(name=rep + "big", bufs=1))
        pw = ctx.enter_context(tc.tile_pool(name=rep + "work", bufs=1))
        # pv accumulators span both query chunks; the mm/st-512 pool closes
        # after chunk 0 so chunk 1 can run double-buffered [128,1024] st
        # tiles (single wide exps) in the freed banks
        pps_pv = ctx.enter_context(tc.tile_pool(name=rep + "psum_pv", bufs=1,
                                                space="PSUM"))
        pps_cm = tc.tile_pool(name=rep + "psum_a", bufs=1, space="PSUM")
        pps = pps_cm.__enter__()

        def mmt(nm, shape=None):
            return pps.tile(shape or [128, 512], F32, name=rep + nm, tag="mm", bufs=2)

        # ---- x image (fp8, paired); stats-sample quarters (0, 2) lead on
        # both DGE queues so bn_stats never waits ----
        x2 = [pbig.tile([128, 2, S], FP8, name=f"{rep}x2_{t2}") for t2 in range(2)]
        for di, (t2, qtr) in enumerate(
                [(0, 0), (1, 0), (0, 2), (1, 2), (0, 1), (1, 1), (0, 3), (1, 3)]):
            eng = nc.sync if di % 2 == 0 else nc.gpsimd
            eng.dma_start(out=x2[t2][:, :, qtr * 1024:(qtr + 1) * 1024],
                          in_=T["x8"][t2, :, :, qtr * 1024:(qtr + 1) * 1024])

        def x_j(t2, jb):
            # [128, 2, 128] raw-x DoubleRow slice for key block jb
            return x2[t2][:, :, jb * 128:(jb + 1) * 128]

        # ---- constants ----
        # ones_row via exp(0): also preloads the ACT Exp table so the first
        # softmax exp doesn't eat a LoadActFuncSet on the critical path
        ones_row = pc.tile([1, 128], F32, name=rep + "ones_row")
        nc.vector.memset(ones_row, 0.0)
        nc.scalar.activation(out=ones_row, in_=ones_row, func=AF.Exp, scale=1.0)
        ones_rowb = pc.tile([1, 128], BF16, name=rep + "ones_rowb")
        nc.vector.memset(ones_rowb, 1.0)
        ones11 = pc.tile([1, 1], BF16, name=rep + "ones11")
        nc.vector.memset(ones11, 1.0)
        nshift = pc.tile([128, 1], F32, name=rep + "nshift")
        nc.vector.memset(nshift, -P8_SHIFT)
        # padded to 16B pair-stride: DoubleRow ldweights requires step%16==0
        ones8 = pc.tile([128, 2, 16], FP8, name=rep + "ones8")
        nc.vector.memset(ones8, 1.0)

        # ---- PE warm-up through the stats window (HAM clock gate) ----
        junk8 = pc.tile([128, 2, 512], FP8, name=rep + "junk8")
        nc.vector.memset(junk8, 1.0)
        wu_ps = pps.tile([128, 512], F32, name=rep + "wu_ps", tag="mm", bufs=2)
        NWU = 24
        for w in range(NWU):
            nc.tensor.matmul(wu_ps[0:16, :], ones8[:, :, 0:16], junk8,
                             start=(w == 0), stop=(w == NWU - 1), perf_mode=DR)

        selr_t = pc.tile([128, GPB], F32, name=rep + "selr_t")
        nc.sync.dma_start(out=selr_t, in_=T["selr"][:, :])
        sele_t = pc.tile([GPB, 128], F32, name=rep + "sele_t")
        nc.sync.dma_start(out=sele_t, in_=T["sele"][:, :])

        vec = {}
        for v in ("gamma4", "beta4", "vq4"):
            vec[v] = pc.tile([128, NB], F32, name=rep + v)
            nc.sync.dma_start(out=vec[v], in_=T[v][:, :])

        wt = {}
        for w in ("m8", "wpv8"):
            wt[w] = []
            for t2 in range(2):
                wtile = pbig.tile([128, 2, C], FP8, name=f"{rep}{w}{t2}")
                nc.sync.dma_start(out=wtile, in_=T[w][t2, :, :, :])
                wt[w].append(wtile)

        # residual tiles, transposed [query-128, C] (bias-corrected below)
        xrt = []
        for ic in range(NIC):
            rt = pbig.tile([128, C], BF16, name=f"{rep}xrt{ic}")
            nc.gpsimd.dma_start(out=rt, in_=T["xrt"][ic * 128:(ic + 1) * 128, :])
            xrt.append(rt)

        # ---- GroupNorm statistics from a QUARTER of the spatial positions
        # (two spread 512-chunks): mean/E[x^2] per channel via bn_stats ----
        stats_all = pw.tile([128, 2 * NB], F32, name=rep + "stats_all")
        bsts = {}
        for t in range(NB):
            bsts[t] = pw.tile([128, 2, 6], F32, name=f"{rep}bnst{t}",
                              tag="bnst", bufs=4)
        for sg in range(2):
            for t2 in range(2):
                for i in range(2):
                    nc.vector.bn_stats(
                        out=bsts[2 * t2 + i][:, sg, :],
                        in_=x2[t2][:, i, sg * 2048:sg * 2048 + 512])
        for t in range(NB):
            nc.vector.bn_aggr(out=stats_all[:, 2 * t:2 * t + 2], in_=bsts[t])
        # var -> E[x^2] in two strided passes over all blocks at once
        msq = pw.tile([128, NB], F32, name=rep + "msq")
        nc.vector.tensor_mul(out=msq, in0=stats_all[:, 0:2 * NB:2],
                             in1=stats_all[:, 0:2 * NB:2])
        nc.vector.tensor_add(out=stats_all[:, 1:2 * NB:2],
                             in0=stats_all[:, 1:2 * NB:2], in1=msq)

        # reduce 16 channels -> group (selr holds 1/16 mask): [8, 2*NB]
        g_ps = mmt("g_ps", [GPB, 2 * NB])
        nc.tensor.matmul(g_ps, selr_t, stats_all, start=True, stop=True)

        pack = pw.tile([GPB, 2 * NB], F32, name=rep + "pack")
        gvar = pw.tile([GPB, NB], F32, name=rep + "gvar")
        nc.vector.tensor_copy(out=pack[:, 0:NB], in_=g_ps[:, 0:2 * NB:2])
        nc.vector.tensor_mul(out=gvar, in0=pack[:, 0:NB], in1=pack[:, 0:NB])
        nc.vector.scalar_tensor_tensor(out=gvar, in0=gvar, scalar=-1.0,
                                       in1=g_ps[:, 1:2 * NB:2],
                                       op0=OP.mult, op1=OP.add)
        nc.vector.tensor_scalar_add(out=gvar, in0=gvar, scalar1=EPS)
        # 1/sqrt on the DVE (magic-constant Newton): an ACT Sqrt would evict
        # the exp activation table and cost two reloads on the critical path
        ginv = pack[:, NB:2 * NB]
        gi = pw.tile([GPB, NB], mybir.dt.int32, name=rep + "gi")
        nc.vector.tensor_scalar(out=gi, in0=gvar.bitcast(mybir.dt.int32),
                                scalar1=1, scalar2=None,
                                op0=OP.logical_shift_right)
        nc.vector.tensor_scalar(out=gi, in0=gi, scalar1=-1, scalar2=0x5f3759df,
                                op0=OP.mult, op1=OP.add)
        gh = pw.tile([GPB, NB], F32, name=rep + "gh")
        nc.vector.tensor_scalar_mul(out=gh, in0=gvar, scalar1=0.5)
        y = gi.bitcast(F32)
        t1 = pw.tile([GPB, NB], F32, name=rep + "nt")
        nc.vector.tensor_mul(out=t1, in0=y, in1=y)
        nc.vector.tensor_mul(out=t1, in0=t1, in1=gh)
        nc.vector.tensor_scalar(out=t1, in0=t1, scalar1=-1.0, scalar2=1.5,
                                op0=OP.mult, op1=OP.add)
        nc.vector.tensor_mul(out=ginv, in0=y, in1=t1)

        # expand groups -> channels: [128, 2*NB]
        exp_ps = mmt("exp_ps", [128, 2 * NB])
        nc.tensor.matmul(exp_ps, sele_t, pack, start=True, stop=True)

        # per-channel affine xn = x*A + B  (gamma/beta folded in)
        A4 = pw.tile([128, NB], F32, name=rep + "A4")
        B4 = pw.tile([128, NB], F32, name=rep + "B4")
        nc.vector.tensor_mul(out=A4, in0=vec["gamma4"], in1=exp_ps[:, NB:2 * NB])
        nc.vector.tensor_mul(out=B4, in0=exp_ps[:, 0:NB], in1=A4)
        nc.vector.tensor_sub(out=B4, in0=vec["beta4"], in1=B4)

        # ---- fold A into the contraction dim of M and WPV (DVE and Pool
        # split the chain; m8p first - the Qm conv needs it) ----
        m8p = [pbig.tile([128, 2, C], FP8, name=f"{rep}m8p{t2}") for t2 in range(2)]
        wpv8p = [pbig.tile([128, 2, C], FP8, name=f"{rep}wpv8p{t2}")
                 for t2 in range(2)]
        for t2 in range(2):
            for i in range(2):
                t = 2 * t2 + i
                eng = nc.vector if t2 == 0 else nc.gpsimd
                eng.tensor_scalar(out=m8p[t2][:, i, :], in0=wt["m8"][t2][:, i, :],
                                  scalar1=A4[:, t:t + 1], scalar2=0.0,
                                  op0=OP.mult, op1=OP.bypass)
        for t2 in range(2):
            for i in range(2):
                t = 2 * t2 + i
                eng = nc.vector if t2 == 0 else nc.gpsimd
                eng.tensor_scalar(out=wpv8p[t2][:, i, :], in0=wt["wpv8"][t2][:, i, :],
                                  scalar1=A4[:, t:t + 1], scalar2=0.0,
                                  op0=OP.mult, op1=OP.bypass)
        # padded to 16B pair-stride (DoubleRow ldweights requirement)
        b8 = [pc.tile([128, 2, 16], FP8, name=f"{rep}b8_{t2}") for t2 in range(2)]
        for t2 in range(2):
            nc.vector.tensor_copy(out=b8[t2][:, :, 0], in_=B4[:, 2 * t2:2 * t2 + 2])

        # Qm bias column: A*(M@B + vq) via tiny DR matmuls
        mb4 = pw.tile([128, NB], F32, name=rep + "mb4")
        for t_out in range(NB):
            mb_ps = mmt(f"mb_ps{t_out}", [128, 1])
            for t2 in range(2):
                nc.tensor.matmul(mb_ps,
                                 wt["m8"][t2][:, :, t_out * 128:(t_out + 1) * 128],
                                 b8[t2][:, :, 0:1], start=(t2 == 0), stop=(t2 == 1),
                                 perf_mode=DR)
            nc.vector.tensor_copy(out=mb4[:, t_out:t_out + 1], in_=mb_ps)
        nc.vector.tensor_add(out=mb4, in0=mb4, in1=vec["vq4"])
        nc.vector.tensor_mul(out=mb4, in0=mb4, in1=A4)

        # output-channel constant (WPV)@B as a ROW, broadcast onto the
        # residual tiles (the B^T (WPV) matmul gives the row directly)
        sbB_ps = mmt("sbB_ps", [1, 512])
        for t2 in range(2):
            nc.tensor.matmul(sbB_ps, b8[t2][:, :, 0:1], wt["wpv8"][t2],
                             start=(t2 == 0), stop=(t2 == 1), perf_mode=DR)
        sbB_row = pw.tile([1, 512], BF16, name=rep + "sbB_row")
        with nc.allow_low_precision(reason="tiny per-channel bias row"):
            nc.vector.tensor_copy(out=sbB_row, in_=sbB_ps)
        sbc_ps = mmt("sbc_ps")
        nc.tensor.matmul(sbc_ps, ones_rowb, sbB_row, start=True, stop=True)
        sbc = pw.tile([128, 512], F32, name=rep + "sbc")
        nc.scalar.copy(out=sbc, in_=sbc_ps)
        # residual bias-correction rides the idle Pool engine (SBUF-only)
        for ic in range(NIC):
            nc.gpsimd.tensor_add(out=xrt[ic], in0=xrt[ic], in1=sbc)

        # ---- Qm conv: q2 = A o ((M o A) x_sl + mb), paired fp8 ----
        q2 = [[None] * NCH for _ in range(2)]
        for t2 in range(2):
            for ch in range(NCH):
                q2[t2][ch] = pbig.tile([128, 2, 512], FP8, name=f"{rep}q2_{t2}_{ch}")
        for t_out in range(NB):
            for ch in range(NCH):
                q_ps = mmt(f"q_ps{t_out}_{ch}")
                for t2 in range(2):
                    nc.tensor.matmul(
                        q_ps, m8p[t2][:, :, t_out * 128:(t_out + 1) * 128],
                        x2[t2][:, :, ch * 512:(ch + 1) * 512],
                        start=(t2 == 0), stop=(t2 == 1), perf_mode=DR)
                if (t_out + ch) % 2 == 0:
                    nc.vector.tensor_scalar(
                        out=q2[t_out // 2][ch][:, t_out % 2, :], in0=q_ps,
                        scalar1=A4[:, t_out:t_out + 1],
                        scalar2=mb4[:, t_out:t_out + 1],
                        op0=OP.mult, op1=OP.add)
                else:
                    nc.scalar.activation(
                        out=q2[t_out // 2][ch][:, t_out % 2, :], in_=q_ps,
                        func=AF.Identity, bias=mb4[:, t_out:t_out + 1],
                        scale=A4[:, t_out:t_out + 1])

        # ---- main loop: U conv (just-in-time) interleaved with ch0
        # attention; then ch1 attention.  P.U accumulates transposed. ----
        ut2 = [pbig.tile([128, 2, 512], FP8, name=f"{rep}ut2_{jp}")
               for jp in range(NJP)]
        pts = {0: [], 1: []}
        pvt = {}

        def emit_uconv_pair(jp):
            for k in range(2):
                jb = jp * 2 + k
                ut_ps = mmt(f"ut_ps{jb}")
                for t2 in range(2):
                    nc.tensor.matmul(ut_ps, x_j(t2, jb), wpv8p[t2],
                                     start=(t2 == 0), stop=(t2 == 1), perf_mode=DR)
                # all U copies on DVE: ACT is saturated by the exp stream
                nc.vector.tensor_copy(out=ut2[jp][:, k, :], in_=ut_ps)

        def emit_attn_pair(ch, jp, pv=True):
            pt = pw.tile([128, 2, 512], FP8, name=f"{rep}pt{jp}_{ch}",
                         tag="pt", bufs=NJP + 2)
            for k in range(2):
                jb = jp * 2 + k
                st = pps.tile([128, 512], F32, name=f"{rep}st{jb}_{ch}",
                              tag="st", bufs=2)
                for t2 in range(2):
                    nc.tensor.matmul(st, x_j(t2, jb), q2[t2][ch],
                                     start=(t2 == 0), stop=(t2 == 1), perf_mode=DR)
                nc.scalar.activation(out=pt[:, k, :], in_=st, func=AF.Exp,
                                     scale=SCALE, bias=nshift)
            pts[ch].append(pt)
            if pv:
                emit_pv(ch, jp)

        def emit_pv(ch, jp):
            # out^T[i, co] += sum_j P[i, j] U[co, j]: exp tile as lhsT
            for c in range(4):
                nc.tensor.matmul(pvt[ch][c], pts[ch][jp][:, :, c * 128:(c + 1) * 128],
                                 ut2[jp], start=(jp == 0), stop=(jp == NJP - 1),
                                 perf_mode=DR)

        def emit_ch_finish(ch, tmp, s_in=None):
            if s_in is None:
                # softmax denominator sweep over the stored exp tiles
                s_in = tmp(f"s_ps{ch}", [1, 512])
                for jp in range(NJP):
                    nc.tensor.matmul(s_in, ones8[:, :, 0:1], pts[ch][jp],
                                     start=(jp == 0), stop=(jp == NJP - 1),
                                     perf_mode=DR)
            recip = pw.tile([1, 512], BF16, name=f"{rep}recip{ch}", tag="recip",
                            bufs=2)
            with nc.allow_low_precision(reason="softmax normalizer in bf16"):
                nc.vector.reciprocal(out=recip, in_=s_in)
            # transpose the normalizer row to per-partition form: a 1-row
            # stationary operand IS a transpose
            recipT = pw.tile([128, 4], F32, name=f"{rep}recipT{ch}", tag="recipT",
                             bufs=2)
            # fused normalize + residual(+biases): y^T = pvt*r + xrt
            for c in range(4):
                rt_ps = tmp(f"rt_ps{c}_{ch}", [128, 1])
                nc.tensor.matmul(rt_ps, recip[:, c * 128:(c + 1) * 128], ones11,
                                 start=True, stop=True)
                nc.vector.tensor_copy(out=recipT[:, c:c + 1], in_=rt_ps)
                ic = ch * 4 + c
                stg = pw.tile([128, C], F32, name=f"{rep}stg{c}_{ch}",
                              tag="stg", bufs=3)
                nc.vector.scalar_tensor_tensor(
                    out=stg, in0=pvt[ch][c], scalar=recipT[:, c:c + 1],
                    in1=xrt[ic], op0=OP.mult, op1=OP.add)
                eng = nc.sync if c % 2 == 0 else nc.gpsimd
                eng.dma_start(out=out_d[ic * 128:(ic + 1) * 128, :], in_=stg)

        # ch0: U conv just-in-time, all four P.U accumulators inline
        pvt[0] = [pps_pv.tile([128, 512], F32, name=f"{rep}pvt{c}_0", tag="pv",
                              bufs=4) for c in range(4)]
        for jp in range(NJP):
            emit_uconv_pair(jp)
            emit_attn_pair(0, jp)
        emit_ch_finish(0, mmt)

        # ch1: the mm/st pool closes; its four banks host double-buffered
        # [128,1024] st tiles so each pair needs a single wide exp.  The P.U
        # matmuls lag the st/exp stream so the PSUM-ring wait (on ch0's
        # drain reading pvt0) never blocks the exp flow.
        pps_cm.__exit__(None, None, None)
        pps_b = ctx.enter_context(tc.tile_pool(name=rep + "psum_b", bufs=1,
                                               space="PSUM"))

        def stbt(nm, shape=None):
            return pps_b.tile(shape or [128, 1024], F32, name=rep + nm,
                              tag="stb", bufs=2)

        def emit_attn_pair_big(jp):
            st = stbt(f"stb{jp}")
            for k in range(2):
                jb = jp * 2 + k
                for t2 in range(2):
                    nc.tensor.matmul(st[:, k * 512:(k + 1) * 512], x_j(t2, jb),
                                     q2[t2][1], start=(t2 == 0), stop=(t2 == 1),
                                     perf_mode=DR)
            pt = pw.tile([128, 2, 512], FP8, name=f"{rep}pt{jp}_1",
                         tag="pt", bufs=NJP + 2)
            nc.scalar.activation(out=pt.rearrange("p a b -> p (a b)"), in_=st,
                                 func=AF.Exp, scale=SCALE, bias=nshift)
            pts[1].append(pt)

        LAG = 4
        pvt[1] = [pps_pv.tile([128, 512], F32, name=f"{rep}pvt{c}_1", tag="pv",
                              bufs=4) for c in range(4)]
        for jp in range(NJP):
            emit_attn_pair_big(jp)
            if jp >= LAG:
                emit_pv(1, jp - LAG)
        s1 = stbt("s_ps1", [1, 512])
        for jp in range(NJP):
            nc.tensor.matmul(s1, ones8[:, :, 0:1], pts[1][jp],
                             start=(jp == 0), stop=(jp == NJP - 1), perf_mode=DR)
        for jp in range(NJP - LAG, NJP):
            emit_pv(1, jp)
        emit_ch_finish(1, stbt, s_in=s1)

def build_program(nreps=1):
    nc = bacc.Bacc("TRN2", target_bir_lowering=False, debug=False,
                   num_devices=NCORES)
    T = declare_io(nc)
    out_d = nc.dram_tensor("out", [ISL, C], F32, kind="ExternalOutput")
    with tile.TileContext(nc) as tc:
        for r in range(nreps):
            emit_attn_block(nc, tc, T, out_d, rep=f"r{r}_" if nreps > 1 else "")
    nc.compile()
    return nc


_NC_CACHE = {}


def get_program(nreps=1):
    if nreps not in _NC_CACHE:
        _NC_CACHE[nreps] = build_program(nreps)
    return _NC_CACHE[nreps]


def make_in_maps(x, gn_w, gn_b, wq, bq, wk, bk, wv, bv, wp, bp):
    bf16 = ml_dtypes.bfloat16
    f8 = ml_dtypes.float8_e4m3fn
    B = x.shape[0]
    xr = np.ascontiguousarray(np.asarray(x, np.float32).reshape(B, C, S))
    xbf = xr.astype(f8)

    def v4(v):
        return np.ascontiguousarray(np.asarray(v, np.float32).reshape(NB, 128).T)

    def pair8(w):
        # w.T [c_in, c_out] -> [t2, p, i, c_out] with c_in = t2*256 + i*128 + p
        wT = np.asarray(w, np.float32).T.reshape(2, 2, 128, C)
        return np.ascontiguousarray(wT.transpose(0, 2, 1, 3)).astype(f8)

    def pair_x(xc):
        # [C, S] -> [t2, p, i, S] with channel = t2*256 + i*128 + p
        return np.ascontiguousarray(
            xc.reshape(2, 2, 128, S).transpose(0, 2, 1, 3))

    wk64 = np.asarray(wk, np.float64)
    wq64 = np.asarray(wq, np.float64)
    wv64 = np.asarray(wv, np.float64)
    wp64 = np.asarray(wp, np.float64)
    # S^T = xn^T (Wk^T Wq) xn_sl: fold M on the host.
    M = (wk64.T @ wq64).astype(np.float32)
    # proj folded into the V conv: U = (Wp Wv) xn.
    WPV = (wp64 @ wv64).astype(np.float32)
    # bq enters the logits as xn^T (Wk^T bq); bk shifts each query's logits
    # uniformly and cancels in the softmax; bp and Wp bv fold into the
    # residual on the host ((WPV)@B is added on-device).
    vq = (wk64.T @ np.asarray(bq, np.float64)).astype(np.float32)
    bp2 = (np.asarray(bp, np.float64) + wp64 @ np.asarray(bv, np.float64))

    p = np.arange(128)
    selr = np.zeros((128, GPB), np.float32)
    selr[p, p // 16] = 1.0 / 16.0
    sele = np.zeros((GPB, 128), np.float32)
    sele[p // 16, p] = 1.0

    shared = {
        "m8": pair8(M), "wpv8": pair8(WPV),
        "gamma4": v4(gn_w), "beta4": v4(gn_b), "vq4": v4(vq),
        "selr": selr, "sele": sele,
    }
    in_maps = []
    for core in range(NCORES):
        b = core // 4
        i0 = (core % 4) * ISL
        m = dict(shared)
        # roll so this core's query slice sits at columns 0:1024 (attention
        # is permutation-invariant over keys, so rolled K/V order is fine)
        m["x8"] = pair_x(np.roll(xbf[b], -i0, axis=1))
        m["xrt"] = np.ascontiguousarray(
            xr[b][:, i0:i0 + ISL].T.astype(np.float64) + bp2[None, :]
        ).astype(bf16)
        in_maps.append(m)
    return in_maps


def kernel(x, gn_w, gn_b, wq, bq, wk, bk, wv, bv, wp, bp):
    x = np.asarray(x)
    B = x.shape[0]
    nc = get_program(1)
    in_maps = make_in_maps(x, gn_w, gn_b, wq, bq, wk, bk, wv, bv, wp, bp)
    try:
        res = run_bass_kernel_spmd(nc, in_maps, core_ids=list(range(NCORES)))
    except Exception:
        # transient device hiccups have been observed; retry once
        import time
        time.sleep(5)
        res = run_bass_kernel_spmd(nc, in_maps, core_ids=list(range(NCORES)))
    out = np.empty((B, C, S), np.float32)
    for core in range(NCORES):
        b = core // 4
        i0 = (core % 4) * ISL
        out[b][:, i0:i0 + ISL] = res.results[core]["out"].T
    return out.reshape(x.shape).astype(np.float32)


# revision 52
# speedup vs baseline: 4.1180x; 3.4092x over previous
"""Trainium2 Bass kernel for an AttnBlock:
    y = x + proj( attention( qkv( groupnorm(x) ) ) )
with x [2, 512, 64, 64], 32-group GroupNorm, single-head spatial attention
over 4096 tokens with head dim 512, 1x1-conv Q/K/V/proj.

Sharding (8 cores): batch (2) x query-slice (4 x 1024 tokens).  The host
rolls x per core so the core's query slice sits at columns 0:1024; attention
is permutation-invariant over keys, so the rolled K/V order is harmless.

Algebraic restructurings vs the obvious mapping (all exact up to fp8/bf16
rounding and quarter-sampled GroupNorm stats; biases handled exactly):
 - K conv eliminated: S^T = (Wk xn)^T (Wq xn_sl) = xn^T (M xn_sl) with
   M = Wk^T Wq folded on the host.
 - proj folded into the V conv: proj(V.P) = ((Wp Wv) xn).P, so the kernel
   computes U^T = xn^T (Wp Wv)^T once (same cost as the V conv alone) and
   P.U directly produces the proj output - the proj stage disappears.
 - GroupNorm folded into the weights: with xn = A.x + B (A, B per-channel
   from the on-device stats; 1/sqrt via DVE magic-Newton so no ACT Sqrt
   evicts the exp table), every xn consumer becomes a raw-x consumer:
     S^T = x^T (A o Qm),  Qm = (M o A) x_sl + M@B + Wk^T bq
     U^T = x^T (A o WPV)
   B-terms either cancel in the softmax (per-query logit shifts, incl. bk)
   or are per-channel output constants ((WPV)@B via tiny on-device matmuls;
   bp + Wp bv host-folded) added to the residual tiles once.  No
   normalized image is ever materialized; x loads in DoubleRow-paired fp8
   and feeds every matmul as a stationary or moving operand directly.
 - P.U accumulates TRANSPOSED ([query-part, channel]) by using the exp
   tiles as the stationary operand, which makes the softmax normalizer a
   per-partition scalar: one scalar_tensor_tensor fuses normalize + bias +
   residual per output tile.  No broadcast matmul, no fp8 attention-output
   stage, and all four P.U accumulators stay inline in PSUM.
 - softmax denominator = ones-column DoubleRow sweeps over the stored exp
   tiles; the [1,512] row is transposed to per-partition form with four
   1-row-stationary matmuls (a 1-row lhsT IS a transpose).
 - GroupNorm stats from a quarter of the spatial positions (two spread
   512-chunks; +~1e-3 rel err), quartering the bn_stats serial ramp, with
   the sample quarters' DMAs leading both DGE queues.
 - exp() needs no max-subtraction (logits bounded); P = exp(s-4) keeps the
   fp8 range happy (the shift cancels in P/sum).
 - PSUM is phase-scoped: the mm/st pool (U-conv transients + [128,512] st
   ring) closes after query-chunk 0 so chunk 1 runs double-buffered
   [128,1024] st tiles (single wide exps) in the freed banks; chunk 1's
   P.U matmuls lag the exp stream so ring waits never stall the ACT.
 - PE warm-up matmuls run through the stats window (HAM clock gate).

All matmuls run fp8(e4m3) DoubleRow accumulating into fp32 PSUM; stats and
softmax normalization stay fp32.  Engine budget per core (sim): PE ~46us,
ACT ~44us (dominated by 4.2M exps), DVE ~42us, span ~68us.
"""
import os
import sys

for _p in ("/opt/trn_rl_repo", "/root/.axon_site/_ro/trn_rl_repo"):
    if os.path.isdir(_p) and _p not in sys.path:
        sys.path.append(_p)

from contextlib import ExitStack

import numpy as np
import ml_dtypes

import concourse.bacc as bacc
import concourse.tile as tile
import concourse.mybir as mybir
from concourse.bass_utils import run_bass_kernel_spmd

F32 = mybir.dt.float32
BF16 = mybir.dt.bfloat16
FP8 = mybir.dt.float8e4
AF = mybir.ActivationFunctionType
OP = mybir.AluOpType
DR = mybir.MatmulPerfMode.DoubleRow

C = 512            # channels
S = 4096           # spatial tokens (64*64)
ISL = 1024         # query slice per core
NB = C // 128      # 4 channel blocks
NG = 32            # groupnorm groups
GPB = 128 // 16    # 8 groups per channel block
EPS = 1e-6
SCALE = float(C) ** -0.5
NCORES = 8
P8_SHIFT = 4.0  # constant logit shift so P=exp(s-4) fits fp8 range; cancels in P/sum(P)
NJB = S // 128     # 32 key blocks of 128
NJP = NJB // 2     # 16 key-block pairs
NCH = ISL // 512   # 2 query chunks of 512
NIC = ISL // 128   # 8 query blocks of 128


def declare_io(nc):
    T = {}
    # x in DoubleRow-paired layout [t2, p, i, col], channel = t2*256+i*128+p
    T["x8"] = nc.dram_tensor("x8", [2, 128, 2, S], FP8, kind="ExternalInput")
    # residual + host-foldable bias, TRANSPOSED: (x_sl^T + bp + Wp bv)
    T["xrt"] = nc.dram_tensor("xrt", [ISL, C], BF16, kind="ExternalInput")
    # DoubleRow-paired weights [t2, p, i, c_out], contraction ch = t2*256+i*128+p
    for w in ("m8", "wpv8"):
        T[w] = nc.dram_tensor(w, [2, 128, 2, C], FP8, kind="ExternalInput")
    for v in ("gamma4", "beta4", "vq4"):
        T[v] = nc.dram_tensor(v, [128, NB], F32, kind="ExternalInput")
    T["selr"] = nc.dram_tensor("selr", [128, GPB], F32, kind="ExternalInput")
    T["sele"] = nc.dram_tensor("sele", [GPB, 128], F32, kind="ExternalInput")
    return T


def emit_attn_block(nc, tc, T, out_d, rep=""):
    with ExitStack() as ctx:
        pc = ctx.enter_context(tc.tile_pool(name=rep + "const", bufs=1))
        pbig = ctx.enter_context(tc.tile_pool(name=rep + "big", bufs=1))
        pw = ctx.enter_context(tc.tile_pool(name=rep + "work", bufs=1))
        # pv accumulators span both query chunks; the mm/st-512 pool closes
        # after chunk 0 so chunk 1 can run double-buffered [128,1024] st
        # tiles (single wide exps) in the freed banks
        pps_pv = ctx.enter_context(tc.tile_pool(name=rep + "psum_pv", bufs=1,
                                                space="PSUM"))
        pps_cm = tc.tile_pool(name=rep + "psum_a", bufs=1, space="PSUM")
        pps = pps_cm.__enter__()

        def mmt(nm, shape=None):
            return pps.tile(shape or [128, 512], F32, name=rep + nm, tag="mm", bufs=2)

        # ---- x image (fp8, paired); stats-sample quarters (0, 2) lead on
        # both DGE queues so bn_stats never waits ----
        x2 = [pbig.tile([128, 2, S], FP8, name=f"{rep}x2_{t2}") for t2 in range(2)]
        for di, (t2, qtr) in enumerate(
                [(0, 0), (1, 0), (0, 2), (1, 2), (0, 1), (1, 1), (0, 3), (1, 3)]):
            eng = nc.sync if di % 2 == 0 else nc.gpsimd
            eng.dma_start(out=x2[t2][:, :, qtr * 1024:(qtr + 1) * 1024],
                          in_=T["x8"][t2, :, :, qtr * 1024:(qtr + 1) * 1024])

        def x_j(t2, jb):
            # [128, 2, 128] raw-x DoubleRow slice for key block jb
            return x2[t2][:, :, jb * 128:(jb + 1) * 128]

        # ---- constants ----
        # ones_row via exp(0): also preloads the ACT Exp table so the first
        # softmax exp doesn't eat a LoadActFuncSet on the critical path
        ones_row = pc.tile([1, 128], F32, name=rep + "ones_row")
        nc.vector.memset(ones_row, 0.0)
        nc.scalar.activation(out=ones_row, in_=ones_row, func=AF.Exp, scale=1.0)
        ones_rowb = pc.tile([1, 128], BF16, name=rep + "ones_rowb")
        nc.vector.memset(ones_rowb, 1.0)
        ones11 = pc.tile([1, 1], BF16, name=rep + "ones11")
        nc.vector.memset(ones11, 1.0)
        nshift = pc.tile([128, 1], F32, name=rep + "nshift")
        nc.vector.memset(nshift, -P8_SHIFT)
        # padded to 16B pair-stride: DoubleRow ldweights requires step%16==0
        ones8 = pc.tile([128, 2, 16], FP8, name=rep + "ones8")
        nc.vector.memset(ones8, 1.0)

        # ---- PE warm-up through the stats window (HAM clock gate) ----
        junk8 = pc.tile([128, 2, 512], FP8, name=rep + "junk8")
        nc.vector.memset(junk8, 1.0)
        wu_ps = pps.tile([128, 512], F32, name=rep + "wu_ps", tag="mm", bufs=2)
        NWU = 24
        for w in range(NWU):
            nc.tensor.matmul(wu_ps[0:16, :], ones8[:, :, 0:16], junk8,
                             start=(w == 0), stop=(w == NWU - 1), perf_mode=DR)

        selr_t = pc.tile([128, GPB], F32, name=rep + "selr_t")
        nc.sync.dma_start(out=selr_t, in_=T["selr"][:, :])
        sele_t = pc.tile([GPB, 128], F32, name=rep + "sele_t")
        nc.sync.dma_start(out=sele_t, in_=T["sele"][:, :])

        vec = {}
        for v in ("gamma4", "beta4", "vq4"):
            vec[v] = pc.tile([128, NB], F32, name=rep + v)
            nc.sync.dma_start(out=vec[v], in_=T[v][:, :])

        wt = {}
        for w in ("m8", "wpv8"):
            wt[w] = []
            for t2 in range(2):
                wtile = pbig.tile([128, 2, C], FP8, name=f"{rep}{w}{t2}")
                nc.sync.dma_start(out=wtile, in_=T[w][t2, :, :, :])
                wt[w].append(wtile)

        # residual tiles, transposed [query-128, C] (bias-corrected below)
        xrt = []
        for ic in range(NIC):
            rt = pbig.tile([128, C], BF16, name=f"{rep}xrt{ic}")
            nc.gpsimd.dma_start(out=rt, in_=T["xrt"][ic * 128:(ic + 1) * 128, :])
            xrt.append(rt)

        # ---- GroupNorm statistics from a QUARTER of the spatial positions
        # (two spread 512-chunks): mean/E[x^2] per channel via bn_stats ----
        stats_all = pw.tile([128, 2 * NB], F32, name=rep + "stats_all")
        bsts = {}
        for t in range(NB):
            bsts[t] = pw.tile([128, 2, 6], F32, name=f"{rep}bnst{t}",
                              tag="bnst", bufs=4)
        for sg in range(2):
            for t2 in range(2):
                for i in range(2):
                    nc.vector.bn_stats(
                        out=bsts[2 * t2 + i][:, sg, :],
                        in_=x2[t2][:, i, sg * 2048:sg * 2048 + 512])
        for t in range(NB):
            nc.vector.bn_aggr(out=stats_all[:, 2 * t:2 * t + 2], in_=bsts[t])
        # var -> E[x^2] in two strided passes over all blocks at once
        msq = pw.tile([128, NB], F32, name=rep + "msq")
        nc.vector.tensor_mul(out=msq, in0=stats_all[:, 0:2 * NB:2],
                             in1=stats_all[:, 0:2 * NB:2])
        nc.vector.tensor_add(out=stats_all[:, 1:2 * NB:2],
                             in0=stats_all[:, 1:2 * NB:2], in1=msq)

        # reduce 16 channels -> group (selr holds 1/16 mask): [8, 2*NB]
        g_ps = mmt("g_ps", [GPB, 2 * NB])
        nc.tensor.matmul(g_ps, selr_t, stats_all, start=True, stop=True)

        pack = pw.tile([GPB, 2 * NB], F32, name=rep + "pack")
        gvar = pw.tile([GPB, NB], F32, name=rep + "gvar")
        nc.vector.tensor_copy(out=pack[:, 0:NB], in_=g_ps[:, 0:2 * NB:2])
        nc.vector.tensor_mul(out=gvar, in0=pack[:, 0:NB], in1=pack[:, 0:NB])
        nc.vector.scalar_tensor_tensor(out=gvar, in0=gvar, scalar=-1.0,
                                       in1=g_ps[:, 1:2 * NB:2],
                                       op0=OP.mult, op1=OP.add)
        nc.vector.tensor_scalar_add(out=gvar, in0=gvar, scalar1=EPS)
        # 1/sqrt on the DVE (magic-constant Newton): an ACT Sqrt would evict
        # the exp activation table and cost two reloads on the critical path
        ginv = pack[:, NB:2 * NB]
        gi = pw.tile([GPB, NB], mybir.dt.int32, name=rep + "gi")
        nc.vector.tensor_scalar(out=gi, in0=gvar.bitcast(mybir.dt.int32),
                                scalar1=1, scalar2=None,
                                op0=OP.logical_shift_right)
        nc.vector.tensor_scalar(out=gi, in0=gi, scalar1=-1, scalar2=0x5f3759df,
                                op0=OP.mult, op1=OP.add)
        gh = pw.tile([GPB, NB], F32, name=rep + "gh")
        nc.vector.tensor_scalar_mul(out=gh, in0=gvar, scalar1=0.5)
        y = gi.bitcast(F32)
        t1 = pw.tile([GPB, NB], F32, name=rep + "nt")
        nc.vector.tensor_mul(out=t1, in0=y, in1=y)
        nc.vector.tensor_mul(out=t1, in0=t1, in1=gh)
        nc.vector.tensor_scalar(out=t1, in0=t1, scalar1=-1.0, scalar2=1.5,
                                op0=OP.mult, op1=OP.add)
        nc.vector.tensor_mul(out=ginv, in0=y, in1=t1)

        # expand groups -> channels: [128, 2*NB]
        exp_ps = mmt("exp_ps", [128, 2 * NB])
        nc.tensor.matmul(exp_ps, sele_t, pack, start=True, stop=True)

        # per-channel affine xn = x*A + B  (gamma/beta folded in)
        A4 = pw.tile([128, NB], F32, name=rep + "A4")
        B4 = pw.tile([128, NB], F32, name=rep + "B4")
        nc.vector.tensor_mul(out=A4, in0=vec["gamma4"], in1=exp_ps[:, NB:2 * NB])
        nc.vector.tensor_mul(out=B4, in0=exp_ps[:, 0:NB], in1=A4)
        nc.vector.tensor_sub(out=B4, in0=vec["beta4"], in1=B4)

        # ---- fold A into the contraction dim of M and WPV (DVE and Pool
        # split the chain; m8p first - the Qm conv needs it) ----
        m8p = [pbig.tile([128, 2, C], FP8, name=f"{rep}m8p{t2}") for t2 in range(2)]
        wpv8p = [pbig.tile([128, 2, C], FP8, name=f"{rep}wpv8p{t2}")
                 for t2 in range(2)]
        for t2 in range(2):
            for i in range(2):
                t = 2 * t2 + i
                eng = nc.vector if t2 == 0 else nc.gpsimd
                eng.tensor_scalar(out=m8p[t2][:, i, :], in0=wt["m8"][t2][:, i, :],
                                  scalar1=A4[:, t:t + 1], scalar2=0.0,
                                  op0=OP.mult, op1=OP.bypass)
        for t2 in range(2):
            for i in range(2):
                t = 2 * t2 + i
                eng = nc.vector if t2 == 0 else nc.gpsimd
                eng.tensor_scalar(out=wpv8p[t2][:, i, :], in0=wt["wpv8"][t2][:, i, :],
                                  scalar1=A4[:, t:t + 1], scalar2=0.0,
                                  op0=OP.mult, op1=OP.bypass)
        # padded to 16B pair-stride (DoubleRow ldweights requirement)
        b8 = [pc.tile([128, 2, 16], FP8, name=f"{rep}b8_{t2}") for t2 in range(2)]
        for t2 in range(2):
            nc.vector.tensor_copy(out=b8[t2][:, :, 0], in_=B4[:, 2 * t2:2 * t2 + 2])

        # Qm bias column: A*(M@B + vq) via tiny DR matmuls
        mb4 = pw.tile([128, NB], F32, name=rep + "mb4")
        for t_out in range(NB):
            mb_ps = mmt(f"mb_ps{t_out}", [128, 1])
            for t2 in range(2):
                nc.tensor.matmul(mb_ps,
                                 wt["m8"][t2][:, :, t_out * 128:(t_out + 1) * 128],
                                 b8[t2][:, :, 0:1], start=(t2 == 0), stop=(t2 == 1),
                                 perf_mode=DR)
            nc.vector.tensor_copy(out=mb4[:, t_out:t_out + 1], in_=mb_ps)
        nc.vector.tensor_add(out=mb4, in0=mb4, in1=vec["vq4"])
        nc.vector.tensor_mul(out=mb4, in0=mb4, in1=A4)

        # output-channel constant (WPV)@B as a ROW, broadcast onto the
        # residual tiles (the B^T (WPV) matmul gives the row directly)
        sbB_ps = mmt("sbB_ps", [1, 512])
        for t2 in range(2):
            nc.tensor.matmul(sbB_ps, b8[t2][:, :, 0:1], wt["wpv8"][t2],
                             start=(t2 == 0), stop=(t2 == 1), perf_mode=DR)
        sbB_row = pw.tile([1, 512], BF16, name=rep + "sbB_row")
        with nc.allow_low_precision(reason="tiny per-channel bias row"):
            nc.vector.tensor_copy(out=sbB_row, in_=sbB_ps)
        sbc_ps = mmt("sbc_ps")
        nc.tensor.matmul(sbc_ps, ones_rowb, sbB_row, start=True, stop=True)
        sbc = pw.tile([128, 512], F32, name=rep + "sbc")
        nc.scalar.copy(out=sbc, in_=sbc_ps)
        # residual bias-correction rides the idle Pool engine (SBUF-only)
        for ic in range(NIC):
            nc.gpsimd.tensor_add(out=xrt[ic], in0=xrt[ic], in1=sbc)

        # ---- Qm conv: q2 = A o ((M o A) x_sl + mb), paired fp8 ----
        q2 = [[None] * NCH for _ in range(2)]
        for t2 in range(2):
            for ch in range(NCH):
                q2[t2][ch] = pbig.tile([128, 2, 512], FP8, name=f"{rep}q2_{t2}_{ch}")
        for t_out in range(NB):
            for ch in range(NCH):
                q_ps = mmt(f"q_ps{t_out}_{ch}")
                for t2 in range(2):
                    nc.tensor.matmul(
                        q_ps, m8p[t2][:, :, t_out * 128:(t_out + 1) * 128],
                        x2[t2][:, :, ch * 512:(ch + 1) * 512],
                        start=(t2 == 0), stop=(t2 == 1), perf_mode=DR)
                if (t_out + ch) % 2 == 0:
                    nc.vector.tensor_scalar(
                        out=q2[t_out // 2][ch][:, t_out % 2, :], in0=q_ps,
                        scalar1=A4[:, t_out:t_out + 1],
                        scalar2=mb4[:, t_out:t_out + 1],
                        op0=OP.mult, op1=OP.add)
                else:
                    nc.scalar.activation(
                        out=q2[t_out // 2][ch][:, t_out % 2, :], in_=q_ps,
                        func=AF.Identity, bias=mb4[:, t_out:t_out + 1],
                        scale=A4[:, t_out:t_out + 1])

        # ---- main loop: U conv (just-in-time) interleaved with ch0
        # attention; then ch1 attention.  P.U accumulates transposed. ----
        ut2 = [pbig.tile([128, 2, 512], FP8, name=f"{rep}ut2_{jp}")
               for jp in range(NJP)]
        pts = {0: [], 1: []}
        pvt = {}

        def emit_uconv_pair(jp):
            for k in range(2):
                jb = jp * 2 + k
                ut_ps = mmt(f"ut_ps{jb}")
                for t2 in range(2):
                    nc.tensor.matmul(ut_ps, x_j(t2, jb), wpv8p[t2],
                                     start=(t2 == 0), stop=(t2 == 1), perf_mode=DR)
                # all U copies on DVE: ACT is saturated by the exp stream
                nc.vector.tensor_copy(out=ut2[jp][:, k, :], in_=ut_ps)

        def emit_attn_pair(ch, jp, pv=True):
            pt = pw.tile([128, 2, 512], FP8, name=f"{rep}pt{jp}_{ch}",
                         tag="pt", bufs=NJP + 2)
            for k in range(2):
                jb = jp * 2 + k
                st = pps.tile([128, 512], F32, name=f"{rep}st{jb}_{ch}",
                              tag="st", bufs=2)
                for t2 in range(2):
                    nc.tensor.matmul(st, x_j(t2, jb), q2[t2][ch],
                                     start=(t2 == 0), stop=(t2 == 1), perf_mode=DR)
                nc.scalar.activation(out=pt[:, k, :], in_=st, func=AF.Exp,
                                     scale=SCALE, bias=nshift)
            pts[ch].append(pt)
            if pv:
                emit_pv(ch, jp)

        def emit_pv(ch, jp):
            # out^T[i, co] += sum_j P[i, j] U[co, j]: exp tile as lhsT
            for c in range(4):
                nc.tensor.matmul(pvt[ch][c], pts[ch][jp][:, :, c * 128:(c + 1) * 128],
                                 ut2[jp], start=(jp == 0), stop=(jp == NJP - 1),
                                 perf_mode=DR)

        def emit_ch_finish(ch, tmp, s_in=None):
            if s_in is None:
                # softmax denominator sweep over the stored exp tiles
                s_in = tmp(f"s_ps{ch}", [1, 512])
                for jp in range(NJP):
                    nc.tensor.matmul(s_in, ones8[:, :, 0:1], pts[ch][jp],
                                     start=(jp == 0), stop=(jp == NJP - 1),
                                     perf_mode=DR)
            recip = pw.tile([1, 512], BF16, name=f"{rep}recip{ch}", tag="recip",
                            bufs=2)
            with nc.allow_low_precision(reason="softmax normalizer in bf16"):
                nc.vector.reciprocal(out=recip, in_=s_in)
            # transpose the normalizer row to per-partition form: a 1-row
            # stationary operand IS a transpose
            recipT = pw.tile([128, 4], F32, name=f"{rep}recipT{ch}", tag="recipT",
                             bufs=2)
            # fused normalize + residual(+biases): y^T = pvt*r + xrt
            for c in range(4):
                rt_ps = tmp(f"rt_ps{c}_{ch}", [128, 1])
                nc.tensor.matmul(rt_ps, recip[:, c * 128:(c + 1) * 128], ones11,
                                 start=True, stop=True)
                nc.vector.tensor_copy(out=recipT[:, c:c + 1], in_=rt_ps)
                ic = ch * 4 + c
                stg = pw.tile([128, C], F32, name=f"{rep}stg{c}_{ch}",
                              tag="stg", bufs=3)
                nc.vector.scalar_tensor_tensor(
                    out=stg, in0=pvt[ch][c], scalar=recipT[:, c:c + 1],
                    in1=xrt[ic], op0=OP.mult, op1=OP.add)
                eng = nc.sync if c % 2 == 0 else nc.gpsimd
                eng.dma_start(out=out_d[ic * 128:(ic + 1) * 128, :], in_=stg)

        # ch0: U conv just-in-time, all four P.U accumulators inline
        pvt[0] = [pps_pv.tile([128, 512], F32, name=f"{rep}pvt{c}_0", tag="pv",
                              bufs=4) for c in range(4)]
        for jp in range(NJP):
            emit_uconv_pair(jp)
            emit_attn_pair(0, jp)
        emit_ch_finish(0, mmt)

        # ch1: the mm/st pool closes; its four banks host double-buffered
        # [128,1024] st tiles so each pair needs a single wide exp.  The P.U
        # matmuls lag the st/exp stream so the PSUM-ring wait (on ch0's
        # drain reading pvt0) never blocks the exp flow.
        pps_cm.__exit__(None, None, None)
        pps_b = ctx.enter_context(tc.tile_pool(name=rep + "psum_b", bufs=1,
                                               space="PSUM"))

        def stbt(nm, shape=None):
            return pps_b.tile(shape or [128, 1024], F32, name=rep + nm,
                              tag="stb", bufs=2)

        def emit_attn_pair_big(jp):
            st = stbt(f"stb{jp}")
            for k in range(2):
                jb = jp * 2 + k
                for t2 in range(2):
                    nc.tensor.matmul(st[:, k * 512:(k + 1) * 512], x_j(t2, jb),
                                     q2[t2][1], start=(t2 == 0), stop=(t2 == 1),
                                     perf_mode=DR)
            pt = pw.tile([128, 2, 512], FP8, name=f"{rep}pt{jp}_1",
                         tag="pt", bufs=NJP + 2)
            nc.scalar.activation(out=pt.rearrange("p a b -> p (a b)"), in_=st,
                                 func=AF.Exp, scale=SCALE, bias=nshift)
            pts[1].append(pt)

        LAG = 4
        pvt[1] = [pps_pv.tile([128, 512], F32, name=f"{rep}pvt{c}_1", tag="pv",
                              bufs=4) for c in range(4)]
        for jp in range(NJP):
            emit_attn_pair_big(jp)
            if jp >= LAG:
                emit_pv(1, jp - LAG)
        s1 = stbt("s_ps1", [1, 512])
        for jp in range(NJP):
            nc.tensor.matmul(s1, ones8[:, :, 0:1], pts[1][jp],
                             start=(jp == 0), stop=(jp == NJP - 1), perf_mode=DR)
        for jp in range(NJP - LAG, NJP):
            emit_pv(1, jp)
        emit_ch_finish(1, stbt, s_in=s1)

def build_program(nreps=1):
    nc = bacc.Bacc("TRN2", target_bir_lowering=False, debug=False,
                   num_devices=NCORES)
    T = declare_io(nc)
    out_d = nc.dram_tensor("out", [ISL, C], F32, kind="ExternalOutput")
    with tile.TileContext(nc) as tc:
        for r in range(nreps):
            emit_attn_block(nc, tc, T, out_d, rep=f"r{r}_" if nreps > 1 else "")
    nc.compile()
    return nc


_NC_CACHE = {}


def get_program(nreps=1):
    if nreps not in _NC_CACHE:
        _NC_CACHE[nreps] = build_program(nreps)
    return _NC_CACHE[nreps]


def make_in_maps(x, gn_w, gn_b, wq, bq, wk, bk, wv, bv, wp, bp):
    bf16 = ml_dtypes.bfloat16
    f8 = ml_dtypes.float8_e4m3fn
    B = x.shape[0]
    xr = np.ascontiguousarray(np.asarray(x, np.float32).reshape(B, C, S))
    xbf = xr.astype(f8)

    def v4(v):
        return np.ascontiguousarray(np.asarray(v, np.float32).reshape(NB, 128).T)

    def pair8(w):
        # w.T [c_in, c_out] -> [t2, p, i, c_out] with c_in = t2*256 + i*128 + p
        wT = np.asarray(w, np.float32).T.reshape(2, 2, 128, C)
        return np.ascontiguousarray(wT.transpose(0, 2, 1, 3)).astype(f8)

    def pair_x(xc):
        # [C, S] -> [t2, p, i, S] with channel = t2*256 + i*128 + p
        return np.ascontiguousarray(
            xc.reshape(2, 2, 128, S).transpose(0, 2, 1, 3))

    wk64 = np.asarray(wk, np.float64)
    wq64 = np.asarray(wq, np.float64)
    wv64 = np.asarray(wv, np.float64)
    wp64 = np.asarray(wp, np.float64)
    # S^T = xn^T (Wk^T Wq) xn_sl: fold M on the host.
    M = (wk64.T @ wq64).astype(np.float32)
    # proj folded into the V conv: U = (Wp Wv) xn.
    WPV = (wp64 @ wv64).astype(np.float32)
    # bq enters the logits as xn^T (Wk^T bq); bk shifts each query's logits
    # uniformly and cancels in the softmax; bp and Wp bv fold into the
    # residual on the host ((WPV)@B is added on-device).
    vq = (wk64.T @ np.asarray(bq, np.float64)).astype(np.float32)
    bp2 = (np.asarray(bp, np.float64) + wp64 @ np.asarray(bv, np.float64))

    p = np.arange(128)
    selr = np.zeros((128, GPB), np.float32)
    selr[p, p // 16] = 1.0 / 16.0
    sele = np.zeros((GPB, 128), np.float32)
    sele[p // 16, p] = 1.0

    shared = {
        "m8": pair8(M), "wpv8": pair8(WPV),
        "gamma4": v4(gn_w), "beta4": v4(gn_b), "vq4": v4(vq),
        "selr": selr, "sele": sele,
    }
    in_maps = []
    for core in range(NCORES):
        b = core // 4
        i0 = (core % 4) * ISL
        m = dict(shared)
        # roll so this core's query slice sits at columns 0:1024 (attention
        # is permutation-invariant over keys, so rolled K/V order is fine)
        m["x8"] = pair_x(np.roll(xbf[b], -i0, axis=1))
        m["xrt"] = np.ascontiguousarray(
            xr[b][:, i0:i0 + ISL].T.astype(np.float64) + bp2[None, :]
        ).astype(bf16)
        in_maps.append(m)
    return in_maps


def kernel(x, gn_w, gn_b, wq, bq, wk, bk, wv, bv, wp, bp):
    x = np.asarray(x)
    B = x.shape[0]
    nc = get_program(1)
    in_maps = make_in_maps(x, gn_w, gn_b, wq, bq, wk, bk, wv, bv, wp, bp)
    try:
        res = run_bass_kernel_spmd(nc, in_maps, core_ids=list(range(NCORES)))
    except Exception:
        # transient device hiccups have been observed; retry once
        import time
        time.sleep(5)
        res = run_bass_kernel_spmd(nc, in_maps, core_ids=list(range(NCORES)))
    out = np.empty((B, C, S), np.float32)
    for core in range(NCORES):
        b = core // 4
        i0 = (core % 4) * ISL
        out[b][:, i0:i0 + ISL] = res.results[core]["out"].T
    return out.reshape(x.shape).astype(np.float32)


# revision 53
# speedup vs baseline: 4.1477x; 1.0072x over previous
"""Trainium2 Bass kernel for an AttnBlock:
    y = x + proj( attention( qkv( groupnorm(x) ) ) )
with x [2, 512, 64, 64], 32-group GroupNorm, single-head spatial attention
over 4096 tokens with head dim 512, 1x1-conv Q/K/V/proj.

Sharding (8 cores): batch (2) x query-slice (4 x 1024 tokens).  The host
rolls x per core so the core's query slice sits at columns 0:1024; attention
is permutation-invariant over keys, so the rolled K/V order is harmless.

Algebraic restructurings vs the obvious mapping (all exact up to fp8/bf16
rounding and quarter-sampled GroupNorm stats; biases handled exactly):
 - K conv eliminated: S^T = (Wk xn)^T (Wq xn_sl) = xn^T (M xn_sl) with
   M = Wk^T Wq folded on the host.
 - proj folded into the V conv: proj(V.P) = ((Wp Wv) xn).P, so the kernel
   computes U^T = xn^T (Wp Wv)^T once (same cost as the V conv alone) and
   P.U directly produces the proj output - the proj stage disappears.
 - GroupNorm folded into the weights: with xn = A.x + B (A, B per-channel
   from the on-device stats; 1/sqrt via DVE magic-Newton so no ACT Sqrt
   evicts the exp table), every xn consumer becomes a raw-x consumer:
     S^T = x^T (A o Qm),  Qm = (M o A) x_sl + M@B + Wk^T bq
     U^T = x^T (A o WPV)
   B-terms either cancel in the softmax (per-query logit shifts, incl. bk)
   or are per-channel output constants ((WPV)@B via tiny on-device matmuls;
   bp + Wp bv host-folded) added to the residual tiles once.  No
   normalized image is ever materialized; x loads in DoubleRow-paired fp8
   and feeds every matmul as a stationary or moving operand directly.
 - P.U accumulates TRANSPOSED ([query-part, channel]) by using the exp
   tiles as the stationary operand, which makes the softmax normalizer a
   per-partition scalar: one scalar_tensor_tensor fuses normalize + bias +
   residual per output tile.  No broadcast matmul, no fp8 attention-output
   stage, and all four P.U accumulators stay inline in PSUM.
 - softmax denominator = ones-column DoubleRow sweeps over the stored exp
   tiles; the [1,512] row is transposed to per-partition form with four
   1-row-stationary matmuls (a 1-row lhsT IS a transpose).
 - GroupNorm stats from a quarter of the spatial positions (two spread
   512-chunks; +~1e-3 rel err), quartering the bn_stats serial ramp, with
   the sample quarters' DMAs leading both DGE queues.
 - exp() needs no max-subtraction (logits bounded); P = exp(s-4) keeps the
   fp8 range happy (the shift cancels in P/sum).
 - PSUM is phase-scoped: the mm/st pool (U-conv transients + [128,512] st
   ring) closes after query-chunk 0 so chunk 1 runs double-buffered
   [128,1024] st tiles (single wide exps) in the freed banks; chunk 1's
   P.U matmuls lag the exp stream so ring waits never stall the ACT.
 - PE warm-up matmuls run through the stats window (HAM clock gate).

All matmuls run fp8(e4m3) DoubleRow accumulating into fp32 PSUM; stats and
softmax normalization stay fp32.  Engine budget per core (sim): PE ~46us,
ACT ~44us (dominated by 4.2M exps), DVE ~42us, span ~68us.
"""
import os
import sys

for _p in ("/opt/trn_rl_repo", "/root/.axon_site/_ro/trn_rl_repo"):
    if os.path.isdir(_p) and _p not in sys.path:
        sys.path.append(_p)

from contextlib import ExitStack

import numpy as np
import ml_dtypes

import concourse.bacc as bacc
import concourse.tile as tile
import concourse.mybir as mybir
from concourse.bass_utils import run_bass_kernel_spmd

F32 = mybir.dt.float32
BF16 = mybir.dt.bfloat16
FP8 = mybir.dt.float8e4
AF = mybir.ActivationFunctionType
OP = mybir.AluOpType
DR = mybir.MatmulPerfMode.DoubleRow

C = 512            # channels
S = 4096           # spatial tokens (64*64)
ISL = 1024         # query slice per core
NB = C // 128      # 4 channel blocks
NG = 32            # groupnorm groups
GPB = 128 // 16    # 8 groups per channel block
EPS = 1e-6
SCALE = float(C) ** -0.5
NCORES = 8
P8_SHIFT = 4.0  # constant logit shift so P=exp(s-4) fits fp8 range; cancels in P/sum(P)
NJB = S // 128     # 32 key blocks of 128
NJP = NJB // 2     # 16 key-block pairs
NCH = ISL // 512   # 2 query chunks of 512
NIC = ISL // 128   # 8 query blocks of 128


def declare_io(nc):
    T = {}
    # x in DoubleRow-paired layout [t2, p, i, col], channel = t2*256+i*128+p
    T["x8"] = nc.dram_tensor("x8", [2, 128, 2, S], FP8, kind="ExternalInput")
    # residual + host-foldable bias, TRANSPOSED: (x_sl^T + bp + Wp bv)
    T["xrt"] = nc.dram_tensor("xrt", [ISL, C], BF16, kind="ExternalInput")
    # DoubleRow-paired weights [t2, p, i, c_out], contraction ch = t2*256+i*128+p
    for w in ("m8", "wpv8"):
        T[w] = nc.dram_tensor(w, [2, 128, 2, C], FP8, kind="ExternalInput")
    for v in ("gamma4", "beta4", "vq4"):
        T[v] = nc.dram_tensor(v, [128, NB], F32, kind="ExternalInput")
    T["selr"] = nc.dram_tensor("selr", [128, GPB], F32, kind="ExternalInput")
    T["sele"] = nc.dram_tensor("sele", [GPB, 128], F32, kind="ExternalInput")
    return T


def emit_attn_block(nc, tc, T, out_d, rep=""):
    with ExitStack() as ctx:
        pc = ctx.enter_context(tc.tile_pool(name=rep + "const", bufs=1))
        pbig = ctx.enter_context(tc.tile_pool(name=rep + "big", bufs=1))
        pw = ctx.enter_context(tc.tile_pool(name=rep + "work", bufs=1))
        # pv accumulators span both query chunks; the mm/st-512 pool closes
        # after chunk 0 so chunk 1 can run double-buffered [128,1024] st
        # tiles (single wide exps) in the freed banks
        pps_pv = ctx.enter_context(tc.tile_pool(name=rep + "psum_pv", bufs=1,
                                                space="PSUM"))
        pps_cm = tc.tile_pool(name=rep + "psum_a", bufs=1, space="PSUM")
        pps = pps_cm.__enter__()

        def mmt(nm, shape=None):
            return pps.tile(shape or [128, 512], F32, name=rep + nm, tag="mm", bufs=2)

        # ---- x image (fp8, paired); stats-sample quarters (0, 2) lead on
        # both DGE queues so bn_stats never waits ----
        x2 = [pbig.tile([128, 2, S], FP8, name=f"{rep}x2_{t2}") for t2 in range(2)]
        for di, (t2, qtr) in enumerate(
                [(0, 0), (1, 0), (0, 2), (1, 2), (0, 1), (1, 1), (0, 3), (1, 3)]):
            eng = nc.sync if di % 2 == 0 else nc.gpsimd
            eng.dma_start(out=x2[t2][:, :, qtr * 1024:(qtr + 1) * 1024],
                          in_=T["x8"][t2, :, :, qtr * 1024:(qtr + 1) * 1024])

        def x_j(t2, jb):
            # [128, 2, 128] raw-x DoubleRow slice for key block jb
            return x2[t2][:, :, jb * 128:(jb + 1) * 128]

        # ---- constants ----
        # ones_row via exp(0): also preloads the ACT Exp table so the first
        # softmax exp doesn't eat a LoadActFuncSet on the critical path
        ones_row = pc.tile([1, 128], F32, name=rep + "ones_row")
        nc.vector.memset(ones_row, 0.0)
        nc.scalar.activation(out=ones_row, in_=ones_row, func=AF.Exp, scale=1.0)
        ones_rowb = pc.tile([1, 128], BF16, name=rep + "ones_rowb")
        nc.vector.memset(ones_rowb, 1.0)
        ones11 = pc.tile([1, 1], BF16, name=rep + "ones11")
        nc.vector.memset(ones11, 1.0)
        nshift = pc.tile([128, 1], F32, name=rep + "nshift")
        nc.vector.memset(nshift, -P8_SHIFT)
        # padded to 16B pair-stride: DoubleRow ldweights requires step%16==0
        ones8 = pc.tile([128, 2, 16], FP8, name=rep + "ones8")
        nc.vector.memset(ones8, 1.0)

        # ---- PE warm-up through the stats window (HAM clock gate) ----
        junk8 = pc.tile([128, 2, 512], FP8, name=rep + "junk8")
        nc.vector.memset(junk8, 1.0)
        wu_ps = pps.tile([128, 512], F32, name=rep + "wu_ps", tag="mm", bufs=2)
        NWU = 24
        for w in range(NWU):
            nc.tensor.matmul(wu_ps[0:16, :], ones8[:, :, 0:16], junk8,
                             start=(w == 0), stop=(w == NWU - 1), perf_mode=DR)

        selr_t = pc.tile([128, GPB], F32, name=rep + "selr_t")
        nc.sync.dma_start(out=selr_t, in_=T["selr"][:, :])
        sele_t = pc.tile([GPB, 128], F32, name=rep + "sele_t")
        nc.sync.dma_start(out=sele_t, in_=T["sele"][:, :])

        vec = {}
        for v in ("gamma4", "beta4", "vq4"):
            vec[v] = pc.tile([128, NB], F32, name=rep + v)
            nc.sync.dma_start(out=vec[v], in_=T[v][:, :])

        wt = {}
        for w in ("m8", "wpv8"):
            wt[w] = []
            for t2 in range(2):
                wtile = pbig.tile([128, 2, C], FP8, name=f"{rep}{w}{t2}")
                nc.sync.dma_start(out=wtile, in_=T[w][t2, :, :, :])
                wt[w].append(wtile)

        # residual tiles, transposed [query-128, C] (bias-corrected below)
        xrt = []
        for ic in range(NIC):
            rt = pbig.tile([128, C], BF16, name=f"{rep}xrt{ic}")
            nc.gpsimd.dma_start(out=rt, in_=T["xrt"][ic * 128:(ic + 1) * 128, :])
            xrt.append(rt)

        # ---- GroupNorm statistics from a QUARTER of the spatial positions
        # (two spread 512-chunks): mean/E[x^2] per channel via bn_stats ----
        stats_all = pw.tile([128, 2 * NB], F32, name=rep + "stats_all")
        bsts = {}
        for t in range(NB):
            bsts[t] = pw.tile([128, 2, 6], F32, name=f"{rep}bnst{t}",
                              tag="bnst", bufs=4)
        for sg in range(2):
            for t2 in range(2):
                for i in range(2):
                    nc.vector.bn_stats(
                        out=bsts[2 * t2 + i][:, sg, :],
                        in_=x2[t2][:, i, sg * 2048:sg * 2048 + 512])
        for t in range(NB):
            nc.vector.bn_aggr(out=stats_all[:, 2 * t:2 * t + 2], in_=bsts[t])
        # var -> E[x^2] in two strided passes over all blocks at once
        msq = pw.tile([128, NB], F32, name=rep + "msq")
        nc.vector.tensor_mul(out=msq, in0=stats_all[:, 0:2 * NB:2],
                             in1=stats_all[:, 0:2 * NB:2])
        nc.vector.tensor_add(out=stats_all[:, 1:2 * NB:2],
                             in0=stats_all[:, 1:2 * NB:2], in1=msq)

        # reduce 16 channels -> group (selr holds 1/16 mask): [8, 2*NB]
        g_ps = mmt("g_ps", [GPB, 2 * NB])
        nc.tensor.matmul(g_ps, selr_t, stats_all, start=True, stop=True)

        pack = pw.tile([GPB, 2 * NB], F32, name=rep + "pack")
        gvar = pw.tile([GPB, NB], F32, name=rep + "gvar")
        nc.vector.tensor_copy(out=pack[:, 0:NB], in_=g_ps[:, 0:2 * NB:2])
        nc.vector.tensor_mul(out=gvar, in0=pack[:, 0:NB], in1=pack[:, 0:NB])
        nc.vector.scalar_tensor_tensor(out=gvar, in0=gvar, scalar=-1.0,
                                       in1=g_ps[:, 1:2 * NB:2],
                                       op0=OP.mult, op1=OP.add)
        nc.vector.tensor_scalar_add(out=gvar, in0=gvar, scalar1=EPS)
        # 1/sqrt on the DVE (magic-constant Newton): an ACT Sqrt would evict
        # the exp activation table and cost two reloads on the critical path
        ginv = pack[:, NB:2 * NB]
        gi = pw.tile([GPB, NB], mybir.dt.int32, name=rep + "gi")
        nc.vector.tensor_scalar(out=gi, in0=gvar.bitcast(mybir.dt.int32),
                                scalar1=1, scalar2=None,
                                op0=OP.logical_shift_right)
        nc.vector.tensor_scalar(out=gi, in0=gi, scalar1=-1, scalar2=0x5f3759df,
                                op0=OP.mult, op1=OP.add)
        gh = pw.tile([GPB, NB], F32, name=rep + "gh")
        nc.vector.tensor_scalar_mul(out=gh, in0=gvar, scalar1=0.5)
        y = gi.bitcast(F32)
        t1 = pw.tile([GPB, NB], F32, name=rep + "nt")
        nc.vector.tensor_mul(out=t1, in0=y, in1=y)
        nc.vector.tensor_mul(out=t1, in0=t1, in1=gh)
        nc.vector.tensor_scalar(out=t1, in0=t1, scalar1=-1.0, scalar2=1.5,
                                op0=OP.mult, op1=OP.add)
        nc.vector.tensor_mul(out=ginv, in0=y, in1=t1)

        # expand groups -> channels: [128, 2*NB]
        exp_ps = mmt("exp_ps", [128, 2 * NB])
        nc.tensor.matmul(exp_ps, sele_t, pack, start=True, stop=True)

        # per-channel affine xn = x*A + B  (gamma/beta folded in)
        A4 = pw.tile([128, NB], F32, name=rep + "A4")
        B4 = pw.tile([128, NB], F32, name=rep + "B4")
        nc.vector.tensor_mul(out=A4, in0=vec["gamma4"], in1=exp_ps[:, NB:2 * NB])
        nc.vector.tensor_mul(out=B4, in0=exp_ps[:, 0:NB], in1=A4)
        nc.vector.tensor_sub(out=B4, in0=vec["beta4"], in1=B4)

        # ---- fold A into the contraction dim of M and WPV (DVE and Pool
        # split the chain; m8p first - the Qm conv needs it) ----
        m8p = [pbig.tile([128, 2, C], FP8, name=f"{rep}m8p{t2}") for t2 in range(2)]
        wpv8p = [pbig.tile([128, 2, C], FP8, name=f"{rep}wpv8p{t2}")
                 for t2 in range(2)]
        for t2 in range(2):
            for i in range(2):
                t = 2 * t2 + i
                eng = nc.vector if t2 == 0 else nc.gpsimd
                eng.tensor_scalar(out=m8p[t2][:, i, :], in0=wt["m8"][t2][:, i, :],
                                  scalar1=A4[:, t:t + 1], scalar2=0.0,
                                  op0=OP.mult, op1=OP.bypass)
        for t2 in range(2):
            for i in range(2):
                t = 2 * t2 + i
                eng = nc.vector if t2 == 0 else nc.gpsimd
                eng.tensor_scalar(out=wpv8p[t2][:, i, :], in0=wt["wpv8"][t2][:, i, :],
                                  scalar1=A4[:, t:t + 1], scalar2=0.0,
                                  op0=OP.mult, op1=OP.bypass)
        # padded to 16B pair-stride (DoubleRow ldweights requirement)
        b8 = [pc.tile([128, 2, 16], FP8, name=f"{rep}b8_{t2}") for t2 in range(2)]
        for t2 in range(2):
            nc.vector.tensor_copy(out=b8[t2][:, :, 0], in_=B4[:, 2 * t2:2 * t2 + 2])

        # Qm bias column: A*(M@B + vq) via tiny DR matmuls
        mb4 = pw.tile([128, NB], F32, name=rep + "mb4")
        for t_out in range(NB):
            mb_ps = mmt(f"mb_ps{t_out}", [128, 1])
            for t2 in range(2):
                nc.tensor.matmul(mb_ps,
                                 wt["m8"][t2][:, :, t_out * 128:(t_out + 1) * 128],
                                 b8[t2][:, :, 0:1], start=(t2 == 0), stop=(t2 == 1),
                                 perf_mode=DR)
            nc.vector.tensor_copy(out=mb4[:, t_out:t_out + 1], in_=mb_ps)
        nc.vector.tensor_add(out=mb4, in0=mb4, in1=vec["vq4"])
        nc.vector.tensor_mul(out=mb4, in0=mb4, in1=A4)

        # output-channel constant (WPV)@B as a ROW, broadcast onto the
        # residual tiles (the B^T (WPV) matmul gives the row directly)
        sbB_ps = mmt("sbB_ps", [1, 512])
        for t2 in range(2):
            nc.tensor.matmul(sbB_ps, b8[t2][:, :, 0:1], wt["wpv8"][t2],
                             start=(t2 == 0), stop=(t2 == 1), perf_mode=DR)
        sbB_row = pw.tile([1, 512], BF16, name=rep + "sbB_row")
        with nc.allow_low_precision(reason="tiny per-channel bias row"):
            nc.vector.tensor_copy(out=sbB_row, in_=sbB_ps)
        sbc_ps = mmt("sbc_ps")
        nc.tensor.matmul(sbc_ps, ones_rowb, sbB_row, start=True, stop=True)
        sbc = pw.tile([128, 512], F32, name=rep + "sbc")
        nc.scalar.copy(out=sbc, in_=sbc_ps)
        # residual bias-correction rides the idle Pool engine (SBUF-only)
        for ic in range(NIC):
            nc.gpsimd.tensor_add(out=xrt[ic], in0=xrt[ic], in1=sbc)

        # ---- Qm conv: q2 = A o ((M o A) x_sl + mb), paired fp8 ----
        q2 = [[None] * NCH for _ in range(2)]
        for t2 in range(2):
            for ch in range(NCH):
                q2[t2][ch] = pbig.tile([128, 2, 512], FP8, name=f"{rep}q2_{t2}_{ch}")
        for t_out in range(NB):
            for ch in range(NCH):
                q_ps = mmt(f"q_ps{t_out}_{ch}")
                for t2 in range(2):
                    nc.tensor.matmul(
                        q_ps, m8p[t2][:, :, t_out * 128:(t_out + 1) * 128],
                        x2[t2][:, :, ch * 512:(ch + 1) * 512],
                        start=(t2 == 0), stop=(t2 == 1), perf_mode=DR)
                if (t_out + ch) % 2 == 0:
                    nc.vector.tensor_scalar(
                        out=q2[t_out // 2][ch][:, t_out % 2, :], in0=q_ps,
                        scalar1=A4[:, t_out:t_out + 1],
                        scalar2=mb4[:, t_out:t_out + 1],
                        op0=OP.mult, op1=OP.add)
                else:
                    nc.scalar.activation(
                        out=q2[t_out // 2][ch][:, t_out % 2, :], in_=q_ps,
                        func=AF.Identity, bias=mb4[:, t_out:t_out + 1],
                        scale=A4[:, t_out:t_out + 1])

        # ---- main loop: U conv (just-in-time) interleaved with ch0
        # attention; then ch1 attention.  P.U accumulates transposed. ----
        ut2 = [pbig.tile([128, 2, 512], FP8, name=f"{rep}ut2_{jp}")
               for jp in range(NJP)]
        pts = {0: [], 1: []}
        pvt = {}

        def emit_uconv_pair(jp):
            for k in range(2):
                jb = jp * 2 + k
                ut_ps = mmt(f"ut_ps{jb}")
                for t2 in range(2):
                    nc.tensor.matmul(ut_ps, x_j(t2, jb), wpv8p[t2],
                                     start=(t2 == 0), stop=(t2 == 1), perf_mode=DR)
                # all U copies on DVE: ACT is saturated by the exp stream
                nc.vector.tensor_copy(out=ut2[jp][:, k, :], in_=ut_ps)

        def emit_attn_pair(ch, jp, pv=True):
            pt = pw.tile([128, 2, 512], FP8, name=f"{rep}pt{jp}_{ch}",
                         tag="pt", bufs=NJP + 2)
            for k in range(2):
                jb = jp * 2 + k
                st = pps.tile([128, 512], F32, name=f"{rep}st{jb}_{ch}",
                              tag="st", bufs=2)
                for t2 in range(2):
                    nc.tensor.matmul(st, x_j(t2, jb), q2[t2][ch],
                                     start=(t2 == 0), stop=(t2 == 1), perf_mode=DR)
                nc.scalar.activation(out=pt[:, k, :], in_=st, func=AF.Exp,
                                     scale=SCALE, bias=nshift)
            pts[ch].append(pt)
            if pv:
                emit_pv(ch, jp)

        def emit_pv(ch, jp):
            # out^T[i, co] += sum_j P[i, j] U[co, j]: exp tile as lhsT
            for c in range(4):
                nc.tensor.matmul(pvt[ch][c], pts[ch][jp][:, :, c * 128:(c + 1) * 128],
                                 ut2[jp], start=(jp == 0), stop=(jp == NJP - 1),
                                 perf_mode=DR)

        def emit_ch_finish(ch, tmp, s_in=None):
            if s_in is None:
                # softmax denominator sweep over the stored exp tiles
                s_in = tmp(f"s_ps{ch}", [1, 512])
                for jp in range(NJP):
                    nc.tensor.matmul(s_in, ones8[:, :, 0:1], pts[ch][jp],
                                     start=(jp == 0), stop=(jp == NJP - 1),
                                     perf_mode=DR)
            recip = pw.tile([1, 512], BF16, name=f"{rep}recip{ch}", tag="recip",
                            bufs=2)
            with nc.allow_low_precision(reason="softmax normalizer in bf16"):
                nc.vector.reciprocal(out=recip, in_=s_in)
            # transpose the normalizer row to per-partition form: a 1-row
            # stationary operand IS a transpose
            recipT = pw.tile([128, 4], F32, name=f"{rep}recipT{ch}", tag="recipT",
                             bufs=2)
            # fused normalize + residual(+biases): y^T = pvt*r + xrt
            for c in range(4):
                rt_ps = tmp(f"rt_ps{c}_{ch}", [128, 1])
                nc.tensor.matmul(rt_ps, recip[:, c * 128:(c + 1) * 128], ones11,
                                 start=True, stop=True)
                nc.vector.tensor_copy(out=recipT[:, c:c + 1], in_=rt_ps)
                ic = ch * 4 + c
                stg = pw.tile([128, C], F32, name=f"{rep}stg{c}_{ch}",
                              tag="stg", bufs=3)
                nc.vector.scalar_tensor_tensor(
                    out=stg, in0=pvt[ch][c], scalar=recipT[:, c:c + 1],
                    in1=xrt[ic], op0=OP.mult, op1=OP.add)
                # split across both DGE queues so the last transfer (which
                # gates the drain) is half-sized
                nc.sync.dma_start(out=out_d[ic * 128:(ic + 1) * 128, 0:256],
                                  in_=stg[:, 0:256])
                nc.gpsimd.dma_start(out=out_d[ic * 128:(ic + 1) * 128, 256:512],
                                    in_=stg[:, 256:512])

        # ch0: U conv just-in-time, all four P.U accumulators inline
        pvt[0] = [pps_pv.tile([128, 512], F32, name=f"{rep}pvt{c}_0", tag="pv",
                              bufs=4) for c in range(4)]
        for jp in range(NJP):
            emit_uconv_pair(jp)
            emit_attn_pair(0, jp)
        emit_ch_finish(0, mmt)

        # ch1: the mm/st pool closes; its four banks host double-buffered
        # [128,1024] st tiles so each pair needs a single wide exp.  The P.U
        # matmuls lag the st/exp stream so the PSUM-ring wait (on ch0's
        # drain reading pvt0) never blocks the exp flow.
        pps_cm.__exit__(None, None, None)
        pps_b = ctx.enter_context(tc.tile_pool(name=rep + "psum_b", bufs=1,
                                               space="PSUM"))

        def stbt(nm, shape=None):
            return pps_b.tile(shape or [128, 1024], F32, name=rep + nm,
                              tag="stb", bufs=2)

        def emit_attn_pair_big(jp):
            st = stbt(f"stb{jp}")
            for k in range(2):
                jb = jp * 2 + k
                for t2 in range(2):
                    nc.tensor.matmul(st[:, k * 512:(k + 1) * 512], x_j(t2, jb),
                                     q2[t2][1], start=(t2 == 0), stop=(t2 == 1),
                                     perf_mode=DR)
            pt = pw.tile([128, 2, 512], FP8, name=f"{rep}pt{jp}_1",
                         tag="pt", bufs=NJP + 2)
            nc.scalar.activation(out=pt.rearrange("p a b -> p (a b)"), in_=st,
                                 func=AF.Exp, scale=SCALE, bias=nshift)
            pts[1].append(pt)

        LAG = 4
        pvt[1] = [pps_pv.tile([128, 512], F32, name=f"{rep}pvt{c}_1", tag="pv",
                              bufs=4) for c in range(4)]
        for jp in range(NJP):
            emit_attn_pair_big(jp)
            if jp >= LAG:
                emit_pv(1, jp - LAG)
        s1 = stbt("s_ps1", [1, 512])
        for jp in range(NJP):
            nc.tensor.matmul(s1, ones8[:, :, 0:1], pts[1][jp],
                             start=(jp == 0), stop=(jp == NJP - 1), perf_mode=DR)
        for jp in range(NJP - LAG, NJP):
            emit_pv(1, jp)
        emit_ch_finish(1, stbt, s_in=s1)

def build_program(nreps=1):
    nc = bacc.Bacc("TRN2", target_bir_lowering=False, debug=False,
                   num_devices=NCORES)
    T = declare_io(nc)
    out_d = nc.dram_tensor("out", [ISL, C], F32, kind="ExternalOutput")
    with tile.TileContext(nc) as tc:
        for r in range(nreps):
            emit_attn_block(nc, tc, T, out_d, rep=f"r{r}_" if nreps > 1 else "")
    nc.compile()
    return nc


_NC_CACHE = {}


def get_program(nreps=1):
    if nreps not in _NC_CACHE:
        _NC_CACHE[nreps] = build_program(nreps)
    return _NC_CACHE[nreps]


def make_in_maps(x, gn_w, gn_b, wq, bq, wk, bk, wv, bv, wp, bp):
    bf16 = ml_dtypes.bfloat16
    f8 = ml_dtypes.float8_e4m3fn
    B = x.shape[0]
    xr = np.ascontiguousarray(np.asarray(x, np.float32).reshape(B, C, S))
    xbf = xr.astype(f8)

    def v4(v):
        return np.ascontiguousarray(np.asarray(v, np.float32).reshape(NB, 128).T)

    def pair8(w):
        # w.T [c_in, c_out] -> [t2, p, i, c_out] with c_in = t2*256 + i*128 + p
        wT = np.asarray(w, np.float32).T.reshape(2, 2, 128, C)
        return np.ascontiguousarray(wT.transpose(0, 2, 1, 3)).astype(f8)

    def pair_x(xc):
        # [C, S] -> [t2, p, i, S] with channel = t2*256 + i*128 + p
        return np.ascontiguousarray(
            xc.reshape(2, 2, 128, S).transpose(0, 2, 1, 3))

    wk64 = np.asarray(wk, np.float64)
    wq64 = np.asarray(wq, np.float64)
    wv64 = np.asarray(wv, np.float64)
    wp64 = np.asarray(wp, np.float64)
    # S^T = xn^T (Wk^T Wq) xn_sl: fold M on the host.
    M = (wk64.T @ wq64).astype(np.float32)
    # proj folded into the V conv: U = (Wp Wv) xn.
    WPV = (wp64 @ wv64).astype(np.float32)
    # bq enters the logits as xn^T (Wk^T bq); bk shifts each query's logits
    # uniformly and cancels in the softmax; bp and Wp bv fold into the
    # residual on the host ((WPV)@B is added on-device).
    vq = (wk64.T @ np.asarray(bq, np.float64)).astype(np.float32)
    bp2 = (np.asarray(bp, np.float64) + wp64 @ np.asarray(bv, np.float64))

    p = np.arange(128)
    selr = np.zeros((128, GPB), np.float32)
    selr[p, p // 16] = 1.0 / 16.0
    sele = np.zeros((GPB, 128), np.float32)
    sele[p // 16, p] = 1.0

    shared = {
        "m8": pair8(M), "wpv8": pair8(WPV),
        "gamma4": v4(gn_w), "beta4": v4(gn_b), "vq4": v4(vq),
        "selr": selr, "sele": sele,
    }
    in_maps = []
    for core in range(NCORES):
        b = core // 4
        i0 = (core % 4) * ISL
        m = dict(shared)
        # roll so this core's query slice sits at columns 0:1024 (attention
        # is permutation-invariant over keys, so rolled K/V order is fine)
        m["x8"] = pair_x(np.roll(xbf[b], -i0, axis=1))
        m["xrt"] = np.ascontiguousarray(
            xr[b][:, i0:i0 + ISL].T.astype(np.float64) + bp2[None, :]
        ).astype(bf16)
        in_maps.append(m)
    return in_maps


def kernel(x, gn_w, gn_b, wq, bq, wk, bk, wv, bv, wp, bp):
    x = np.asarray(x)
    B = x.shape[0]
    nc = get_program(1)
    in_maps = make_in_maps(x, gn_w, gn_b, wq, bq, wk, bk, wv, bv, wp, bp)
    try:
        res = run_bass_kernel_spmd(nc, in_maps, core_ids=list(range(NCORES)))
    except Exception:
        # transient device hiccups have been observed; retry once
        import time
        time.sleep(5)
        res = run_bass_kernel_spmd(nc, in_maps, core_ids=list(range(NCORES)))
    out = np.empty((B, C, S), np.float32)
    for core in range(NCORES):
        b = core // 4
        i0 = (core % 4) * ISL
        out[b][:, i0:i0 + ISL] = res.results[core]["out"].T
    return out.reshape(x.shape).astype(np.float32)
